# revision 20
# baseline (speedup 1.0000x reference)
"""Trainium2 Bass kernel for nn_Attention_4930622456197.

Multi-head causal attention (B=2, S=2048, D=2048, 32 heads x head_dim 64)
with QKVO projections, tensor-parallel over heads across 8 NeuronCores
(4 heads per core).

Per-core plan (all matmul inputs bf16, f32 PSUM accumulation):
  Phase 1  QKV projections from host-pretransposed x^T [D, T]:
           Q^T, K^T in [128 (=2 heads x 64 dims), group, T] layout;
           V in natural [tok, head, 65] layout with a ones column
           appended (row 64 of V_aug.T) so the P@V matmul also produces
           the softmax denominators for free.
  Phase 2  Flash-style causal attention in score-transposed layout
           S^T[s, q] (scores never touch HBM).  exp on ScalarE with the
           1/sqrt(hd) scale folded in; no max-subtraction (scores are
           O(+-8) here, exp is safe in fp32->bf16).  The diagonal
           128x128 block of each strip is masked post-exp with a
           precomputed upper-triangular 0/1 tile.  O^T accumulates in
           PSUM over k-tiles; the 64 ones-columns of V_aug broadcast the
           softmax denominator to PSUM partitions 64..127, and 1/den is
           computed as exp(-ln(den)) on ScalarE.
  Phase 3  Row-parallel output projection producing a partial
           out^T [D, T]; host sums the 8 partials, adds wo_b.

  Emission interleaves the three phases: QKV chunks and output-projection
  tiles are "filler" thunks pumped between attention j-steps, keeping the
  PE array dense (HAM stays at 2.4 GHz) while ScalarE runs the softmax
  exps of the two in-flight head-pair streams.

The harness calls kernel(**inputs) with the full (unsharded) inputs and
expects the full [2, 2048, 2048] float32 output.
"""

import numpy as np
import ml_dtypes

BSZ, SEQ, DIM, NH = 2, 2048, 2048, 32
HD = DIM // NH            # 64
NCORES = 8
HPC = NH // NCORES        # 4 heads per core
HSL = HPC * HD            # 256 head-dims per core
T = BSZ * SEQ             # 4096 flattened tokens
SCALE = 1.0 / float(np.sqrt(HD))
BF16 = ml_dtypes.bfloat16

NKT = DIM // 128          # 16 contraction tiles over model dim
NCH = T // 512            # 8 token chunks of 512
NJ = SEQ // 128           # 16 k-tiles per sequence
NCK = SEQ // 512          # 4 q-chunks per sequence

# Output partial dtype: float32 is safest for the cross-core sum;
# bfloat16 halves the output DMA traffic.
OUT_BF16 = True

LAST_RESULTS = None       # BassKernelResults of the most recent run (for test.py)


# This walrus build caps EVERY instruction (HW-decoded and sequencer alike)
# at one sync-wait, so the legalizer splits excess waits regardless of opcode.
_SEQ_OPCODES = set()
_wc_counter = [0]


def _legalize_bir_waits(bir_bytes):
    """This container's walrus accepts only ONE sync-wait on HW-decoded
    instruction structs ("Too many sync wait commands" otherwise), but Tile
    freely emits 2-3 waits per instruction.  Split excess waits into
    standalone same-engine EventSemaphore instructions placed immediately
    before the instruction — the sequencer executes them in order, so the
    dependency semantics are identical."""
    import json as _json

    d = _json.loads(bir_bytes)
    n_split = 0
    for f in d.get("functions", []):
        for blk in f.get("blocks", []):
            out = []
            for ins in blk.get("instructions", []):
                si = ins.get("sync_info")
                waits = (si or {}).get("on_wait") or []
                if si is not None and len(waits) > 1 and \
                        ins.get("opcode") not in _SEQ_OPCODES:
                    for w in waits[:-1]:
                        _wc_counter[0] += 1
                        out.append({
                            "debug": ins.get("debug", 0),
                            "engine": ins["engine"],
                            "ins": [], "outs": [],
                            "name": f"I-wc{_wc_counter[0]}",
                            "opcode": "EventSemaphore",
                            "sync_info": {"on_wait": [w], "on_update": []},
                        })
                        n_split += 1
                    si["on_wait"] = waits[-1:]
                out.append(ins)
            blk["instructions"] = out
    if n_split:
        print(f"[kernel] wait-legalizer: split {n_split} excess waits")
    return _json.dumps(d).encode()


_hook_installed = [False]


def _install_compile_hook():
    """Route every BIR->NEFF compile in this process through the wait
    legalizer (both the direct bass_utils path and the bass2jax/axon path)."""
    if _hook_installed[0]:
        return
    import concourse.bass_utils as bu

    orig = bu.compile_bir_kernel

    def patched(bir_json, tmpdir, neff_name="file.neff"):
        return orig(_legalize_bir_waits(bir_json), tmpdir, neff_name=neff_name)

    bu.compile_bir_kernel = patched
    try:
        import concourse.bass2jax as b2j
        b2j.compile_bir_kernel = patched
    except Exception:
        pass
    _hook_installed[0] = True


def _build(mask_mode, use_qkb, use_vb, phases=(1, 2, 3)):
    """Builds the Bass program. mask_mode: 'causal' | 'none' | 'general'.
    phases: debug knob to emit only a subset of the pipeline."""
    import concourse.bass as bass
    import concourse.mybir as mybir
    import concourse.tile as tile
    from concourse.masks import make_upper_triangular

    dt = mybir.dt
    f32 = dt.float32
    bf16 = dt.bfloat16
    Exp = mybir.ActivationFunctionType.Exp
    Ln = mybir.ActivationFunctionType.Ln
    Identity = mybir.ActivationFunctionType.Identity
    out_dt = bf16 if OUT_BF16 else f32

    causal = mask_mode == "causal"

    nc = bass.Bass()
    xT_d = nc.dram_tensor("xt", [DIM, T], bf16, kind="ExternalInput")
    wqT_d = nc.dram_tensor("wqt", [DIM, HSL], bf16, kind="ExternalInput")
    wkT_d = nc.dram_tensor("wkt", [DIM, HSL], bf16, kind="ExternalInput")
    wvT_d = nc.dram_tensor("wvt", [DIM, HSL], bf16, kind="ExternalInput")
    woT_d = nc.dram_tensor("wot", [HSL, DIM], bf16, kind="ExternalInput")
    outT_d = nc.dram_tensor("outT", [DIM, T], out_dt, kind="ExternalOutput")
    qb_d = kb_d = vb_d = maskT_d = None
    if use_qkb:
        qb_d = nc.dram_tensor("qb", [HSL], f32, kind="ExternalInput")
        kb_d = nc.dram_tensor("kb", [HSL], f32, kind="ExternalInput")
    if use_vb:
        vb_d = nc.dram_tensor("vb", [HSL], f32, kind="ExternalInput")
    if mask_mode == "general":
        maskT_d = nc.dram_tensor("maskt", [SEQ, SEQ], f32, kind="ExternalInput")

    # 3-D views with 128-partition-major layout
    xT_ap = xT_d[:].rearrange("(kt p) t -> p kt t", p=128)
    wq_ap = wqT_d[:].rearrange("(kt p) m -> p kt m", p=128)
    wk_ap = wkT_d[:].rearrange("(kt p) m -> p kt m", p=128)
    wv_ap = wvT_d[:].rearrange("(kt p) m -> p kt m", p=128)
    wo_ap = woT_d[:].rearrange("(g p) n -> p g n", p=128)
    outT_ap = outT_d[:].rearrange("(ot p) t -> p ot t", p=128)

    with tile.TileContext(nc) as tc:
        with (
            tc.tile_pool(name="singles", bufs=1) as singles,
            tc.tile_pool(name="xload", bufs=3) as xload,
            tc.tile_pool(name="work", bufs=4) as work,
            tc.tile_pool(name="outp", bufs=4) as outp,
            tc.tile_pool(name="psum", bufs=2, space="PSUM") as psum,
            tc.tile_pool(name="otps", bufs=4, space="PSUM") as otps,
        ):
            # ---- resident tensors -------------------------------------
            wq_sb = singles.tile([128, NKT, HSL], bf16)
            wk_sb = singles.tile([128, NKT, HSL], bf16)
            wv_sb = singles.tile([128, NKT, HSL], bf16)
            wo_sb = singles.tile([128, 2, DIM], bf16)
            nc.sync.dma_start(out=wq_sb, in_=wq_ap)
            nc.sync.dma_start(out=wk_sb, in_=wk_ap)
            nc.sync.dma_start(out=wv_sb, in_=wv_ap)
            nc.sync.dma_start(out=wo_sb, in_=wo_ap)

            qt_sb = singles.tile([128, 2, T], bf16)
            kt_sb = singles.tile([128, 2, T], bf16)
            ctxT_sb = singles.tile([128, 2, T], bf16)
            # V with 64 ones-columns per head: the P@V matmul then writes the
            # softmax denominator to PSUM partitions 64..127 (a free
            # cross-partition broadcast).
            vaug_sb = singles.tile([128, T // 128, HPC, 2 * HD], bf16)
            nc.vector.memset(vaug_sb, 1.0)

            qb_sb = kb_sb = vb_bc = None
            if use_qkb:
                qb_sb = singles.tile([128, 2], f32)
                kb_sb = singles.tile([128, 2], f32)
                nc.sync.dma_start(out=qb_sb, in_=qb_d[:].rearrange("(g p) -> p g", p=128))
                nc.sync.dma_start(out=kb_sb, in_=kb_d[:].rearrange("(g p) -> p g", p=128))
            if use_vb:
                vb_bc = singles.tile([128, HSL], f32)
                nc.sync.dma_start(out=vb_bc, in_=vb_d[:].to_broadcast([128, HSL]))

            triu_sb = None
            if causal:
                triu_sb = singles.tile([128, 128], bf16)
                make_upper_triangular(nc, triu_sb, val=1.0, diag=True)

            # ---- emission units ---------------------------------------
            # QKV projections and the output projection are emitted as
            # "filler" thunks interleaved between attention j-steps, so PE
            # always has independent matmul work while ScalarE runs the
            # softmax exps of the in-flight attention streams.

            def qkv_thunks(ch):
                tsl = slice(ch * 512, (ch + 1) * 512)
                xt_box = []

                def load():
                    xt_ch = xload.tile([128, NKT, 512], bf16, tag="xt")
                    nc.sync.dma_start(out=xt_ch, in_=xT_ap[:, :, tsl])
                    xt_box.append(xt_ch)
                yield load

                def qk_group(w_sb, dst_sb, b_sb, g):
                    ps = psum.tile([128, 512], f32, tag="st2", name="qk_ps")
                    for k in range(NKT):
                        nc.tensor.matmul(
                            ps, lhsT=w_sb[:, k, g * 128:(g + 1) * 128],
                            rhs=xt_box[0][:, k, :],
                            start=(k == 0), stop=(k == NKT - 1))
                    if b_sb is not None:
                        nc.scalar.activation(
                            out=dst_sb[:, g, tsl], in_=ps, func=Identity,
                            bias=b_sb[:, g:g + 1], scale=1.0)
                    else:
                        nc.vector.tensor_copy(out=dst_sb[:, g, tsl], in_=ps)

                def v_group(tt):
                    tglob = ch * 4 + tt
                    vps = psum.tile([128, HSL], f32, tag="st2", name="v_ps")
                    for k in range(NKT):
                        nc.tensor.matmul(
                            vps, lhsT=xt_box[0][:, k, tt * 128:(tt + 1) * 128],
                            rhs=wv_sb[:, k, :],
                            start=(k == 0), stop=(k == NKT - 1))
                    vdst = vaug_sb[:, tglob, :, 0:HD]
                    vsrc = vps.rearrange("p (h m) -> p h m", h=HPC)
                    if vb_bc is not None:
                        nc.vector.tensor_add(
                            out=vdst, in0=vsrc,
                            in1=vb_bc.rearrange("p (h m) -> p h m", h=HPC))
                    else:
                        nc.vector.tensor_copy(out=vdst, in_=vsrc)

                import functools
                for (w_sb, dst_sb, b_sb) in ((wq_sb, qt_sb, qb_sb),
                                             (wk_sb, kt_sb, kb_sb)):
                    for g in range(2):
                        yield functools.partial(qk_group, w_sb, dst_sb, b_sb, g)
                for tt in range(4):
                    yield functools.partial(v_group, tt)

            def oproj_thunks(ch):
                import functools
                tsl = slice(ch * 512, (ch + 1) * 512)

                def o_unit(o):
                    ops = psum.tile([128, 512], f32, tag="st2", name="o_ps")
                    for g2 in range(2):
                        nc.tensor.matmul(
                            ops, lhsT=wo_sb[:, g2, o * 128:(o + 1) * 128],
                            rhs=ctxT_sb[:, g2, tsl],
                            start=(g2 == 0), stop=(g2 == 1))
                    osb = outp.tile([128, 512], out_dt, tag="out_sb")
                    if o % 2 == 0:
                        nc.vector.tensor_copy(out=osb, in_=ops)
                    else:
                        nc.scalar.copy(out=osb, in_=ops)
                    nc.sync.dma_start(out=outT_ap[:, o, tsl], in_=osb)

                for o in range(DIM // 128):
                    yield functools.partial(o_unit, o)

            def pump(filler, n=1):
                for _ in range(n):
                    t = next(filler, None)
                    if t is None:
                        return False
                    t()
                return True

            def att_region(b, c, filler):
                """Attention for one (batch, q-chunk): head-pair streams g=0,1
                interleaved per j-step; O^T matmuls lag 2 steps; one filler
                thunk per j-step keeps PE busy during the exps."""
                ots = {}
                for gg in range(2):
                    ots[gg, 0] = otps.tile([128, 512], f32, tag="ot", name="otA")
                    ots[gg, 1] = otps.tile([128, 512], f32, tag="ot", name="otB")
                jmax = 4 * c + 4 if causal else NJ
                pend = []

                def flush_ot(gg, j, qo, pt2):
                    for hh in range(2):
                        nc.tensor.matmul(
                            ots[gg, hh][:, qo:512],
                            lhsT=vaug_sb[:, b * NJ + j, 2 * gg + hh, :],
                            rhs=pt2[:, 512 * hh + qo:512 * hh + 512],
                            start=(j == 0), stop=(j == jmax - 1))

                for j in range(jmax):
                    qo = max(0, j * 128 - c * 512) if causal else 0
                    ssl = slice(b * SEQ + j * 128, b * SEQ + (j + 1) * 128)
                    qsl = slice(b * SEQ + c * 512 + qo, b * SEQ + (c + 1) * 512)
                    for gg in range(2):
                        st2 = psum.tile([128, 1024], f32, tag="st2", name="st2")
                        nc.tensor.matmul(
                            st2[:, qo:512], lhsT=kt_sb[0:64, gg, ssl],
                            rhs=qt_sb[0:64, gg, qsl],
                            start=True, stop=True, tile_position=(0, 0))
                        nc.tensor.matmul(
                            st2[:, 512 + qo:1024], lhsT=kt_sb[64:128, gg, ssl],
                            rhs=qt_sb[64:128, gg, qsl],
                            start=True, stop=True, tile_position=(64, 0))
                        if maskT_d is not None:
                            mt = work.tile([128, 512], f32, tag="mt")
                            nc.sync.dma_start(
                                out=mt,
                                in_=maskT_d[j * 128:(j + 1) * 128,
                                            c * 512:(c + 1) * 512])
                            for hh in range(2):
                                sl = slice(512 * hh, 512 * hh + 512)
                                nc.vector.tensor_add(
                                    out=st2[:, sl], in0=st2[:, sl], in1=mt)
                        pt2 = work.tile([128, 1024], bf16, tag="pt", bufs=6)
                        nc.scalar.activation(
                            out=pt2.rearrange("p (two n) -> p two n", two=2)[:, :, qo:512],
                            in_=st2.rearrange("p (two n) -> p two n", two=2)[:, :, qo:512],
                            func=Exp, scale=SCALE)
                        if causal and j * 128 >= c * 512:
                            dv = pt2.rearrange("p (two n) -> p two n", two=2)[:, :, qo:qo + 128]
                            nc.vector.tensor_mul(
                                out=dv, in0=dv,
                                in1=triu_sb[:, None, :].to_broadcast([128, 2, 128]))
                        pend.append((gg, j, qo, pt2))
                        while len(pend) > 4:
                            flush_ot(*pend.pop(0))
                    pump(filler, 2)
                while pend:
                    flush_ot(*pend.pop(0))
                # chunk end: one f32 copy frees each accumulator slot; the
                # Ln/Exp reciprocal + multiply then run from SBUF overlapped
                # with the next region.
                for gg in range(2):
                    csl = slice(b * SEQ + c * 512, b * SEQ + (c + 1) * 512)
                    for hh in range(2):
                        ot = ots[gg, hh]
                        un = work.tile([128, 512], f32, tag="unctx")
                        nc.vector.tensor_copy(out=un, in_=ot)
                        rb = work.tile([64, 512], f32, tag="rb")
                        nc.scalar.activation(out=rb, in_=un[HD:2 * HD, :],
                                             func=Ln, scale=1.0)
                        nc.scalar.activation(out=rb, in_=rb,
                                             func=Exp, scale=-1.0)
                        nc.vector.tensor_mul(
                            out=ctxT_sb[hh * 64:(hh + 1) * 64, gg, csl],
                            in0=un[0:HD, :], in1=rb)

            # ---- schedule ---------------------------------------------
            from itertools import chain

            def drain(filler):
                while pump(filler):
                    pass

            if 1 in phases and 2 in phases and 3 in phases:
                drain(iter(qkv_thunks(0)))
                regions = [
                    (0, 0, qkv_thunks(1)),
                    (0, 1, qkv_thunks(2)),
                    (0, 2, qkv_thunks(3)),
                    (0, 3, chain(qkv_thunks(4), qkv_thunks(5))),
                    (1, 0, qkv_thunks(6)),
                    (1, 1, qkv_thunks(7)),
                    (1, 2, chain(oproj_thunks(0), oproj_thunks(1),
                                 oproj_thunks(5))),
                    (1, 3, chain(oproj_thunks(2), oproj_thunks(3),
                                 oproj_thunks(4), oproj_thunks(6))),
                ]
                for b, c, filler in regions:
                    filler = iter(filler)
                    att_region(b, c, filler)
                    drain(filler)
                drain(iter(oproj_thunks(7)))
            else:
                # debug path: sequential phases
                if 1 in phases:
                    for ch in range(NCH):
                        drain(iter(qkv_thunks(ch)))
                if 2 in phases:
                    empty = iter(())
                    for b in range(BSZ):
                        for c in range(NCK):
                            att_region(b, c, empty)
                if 3 in phases:
                    for ch in range(NCH):
                        drain(iter(oproj_thunks(ch)))

    return nc


def _classify_mask(mask):
    m = np.asarray(mask, dtype=np.float32).reshape(SEQ, SEQ)
    if not np.any(m):
        return "none", None
    lower_ok = not np.any(m[np.tril_indices(SEQ)])
    upper = m[np.triu_indices(SEQ, 1)]
    if lower_ok and np.all(np.isneginf(upper)):
        return "causal", None
    return "general", np.ascontiguousarray(m.T)


def kernel(x, start_pos, freqs_cis, mask, wq_w, wq_b, wk_w, wk_b,
           wv_w, wv_b, wo_w, wo_b):
    global LAST_RESULTS
    _install_compile_hook()
    from concourse.bass_utils import run_bass_kernel_spmd

    x = np.asarray(x, dtype=np.float32)
    mask_mode, maskT = _classify_mask(mask)
    wq_b = np.asarray(wq_b, dtype=np.float32)
    wk_b = np.asarray(wk_b, dtype=np.float32)
    wv_b = np.asarray(wv_b, dtype=np.float32)
    wo_b = np.asarray(wo_b, dtype=np.float32)
    use_qkb = bool(np.any(wq_b) or np.any(wk_b))
    use_vb = bool(np.any(wv_b))

    nc = _build(mask_mode, use_qkb, use_vb)

    xT = np.ascontiguousarray(x.reshape(T, DIM).T).astype(BF16)
    wqT = np.asarray(wq_w, dtype=np.float32).T.astype(BF16)  # [D, D]
    wkT = np.asarray(wk_w, dtype=np.float32).T.astype(BF16)
    wvT = np.asarray(wv_w, dtype=np.float32).T.astype(BF16)
    wo = np.asarray(wo_w, dtype=np.float32)

    in_maps = []
    for c in range(NCORES):
        sl = slice(HSL * c, HSL * (c + 1))
        im = {
            "xt": xT,
            "wqt": np.ascontiguousarray(wqT[:, sl]),
            "wkt": np.ascontiguousarray(wkT[:, sl]),
            "wvt": np.ascontiguousarray(wvT[:, sl]),
            "wot": np.ascontiguousarray(wo[:, sl].T).astype(BF16),
        }
        if use_qkb:
            im["qb"] = np.ascontiguousarray(wq_b[sl])
            im["kb"] = np.ascontiguousarray(wk_b[sl])
        if use_vb:
            im["vb"] = np.ascontiguousarray(wv_b[sl])
        if mask_mode == "general":
            im["maskt"] = maskT
        in_maps.append(im)

    res = run_bass_kernel_spmd(nc, in_maps, core_ids=list(range(NCORES)))
    LAST_RESULTS = res

    acc = np.zeros((DIM, T), dtype=np.float32)
    for r in res.results:
        acc += np.asarray(r["outT"], dtype=np.float32)
    out = acc.T + wo_b[None, :]
    return out.reshape(BSZ, SEQ, DIM).astype(np.float32)


# revision 21
# speedup vs baseline: 1.0599x; 1.0599x over previous
"""Trainium2 Bass kernel for nn_Attention_4930622456197.

Multi-head causal attention (B=2, S=2048, D=2048, 32 heads x head_dim 64)
with QKVO projections, tensor-parallel over heads across 8 NeuronCores
(4 heads per core).

Per-core plan (all matmul inputs bf16, f32 PSUM accumulation):
  Phase 1  QKV projections from host-pretransposed x^T [D, T]:
           Q^T, K^T in [128 (=2 heads x 64 dims), group, T] layout;
           V in natural [tok, head, 65] layout with a ones column
           appended (row 64 of V_aug.T) so the P@V matmul also produces
           the softmax denominators for free.
  Phase 2  Flash-style causal attention in score-transposed layout
           S^T[s, q] (scores never touch HBM).  exp on ScalarE with the
           1/sqrt(hd) scale folded in; no max-subtraction (scores are
           O(+-8) here, exp is safe in fp32->bf16).  The diagonal
           128x128 block of each strip is masked post-exp with a
           precomputed upper-triangular 0/1 tile.  O^T accumulates in
           PSUM over k-tiles; the 64 ones-columns of V_aug broadcast the
           softmax denominator to PSUM partitions 64..127, and 1/den is
           computed as exp(-ln(den)) on ScalarE.
  Phase 3  Row-parallel output projection producing a partial
           out^T [D, T]; host sums the 8 partials, adds wo_b.

  Emission interleaves the three phases: QKV chunks and output-projection
  tiles are "filler" thunks pumped between attention j-steps, keeping the
  PE array dense (HAM stays at 2.4 GHz) while ScalarE runs the softmax
  exps of the two in-flight head-pair streams.

The harness calls kernel(**inputs) with the full (unsharded) inputs and
expects the full [2, 2048, 2048] float32 output.
"""

import numpy as np
import ml_dtypes

BSZ, SEQ, DIM, NH = 2, 2048, 2048, 32
HD = DIM // NH            # 64
NCORES = 8
HPC = NH // NCORES        # 4 heads per core
HSL = HPC * HD            # 256 head-dims per core
T = BSZ * SEQ             # 4096 flattened tokens
SCALE = 1.0 / float(np.sqrt(HD))
BF16 = ml_dtypes.bfloat16

NKT = DIM // 128          # 16 contraction tiles over model dim
NCH = T // 512            # 8 token chunks of 512
NJ = SEQ // 128           # 16 k-tiles per sequence
NCK = SEQ // 512          # 4 q-chunks per sequence

# Output partial dtype: float32 is safest for the cross-core sum;
# bfloat16 halves the output DMA traffic.
OUT_BF16 = True

LAST_RESULTS = None       # BassKernelResults of the most recent run (for test.py)


# This walrus build caps EVERY instruction (HW-decoded and sequencer alike)
# at one sync-wait, so the legalizer splits excess waits regardless of opcode.
_SEQ_OPCODES = set()
_wc_counter = [0]


def _legalize_bir_waits(bir_bytes):
    """This container's walrus accepts only ONE sync-wait on HW-decoded
    instruction structs ("Too many sync wait commands" otherwise), but Tile
    freely emits 2-3 waits per instruction.  Split excess waits into
    standalone same-engine EventSemaphore instructions placed immediately
    before the instruction — the sequencer executes them in order, so the
    dependency semantics are identical."""
    import json as _json

    d = _json.loads(bir_bytes)
    n_split = 0
    for f in d.get("functions", []):
        for blk in f.get("blocks", []):
            out = []
            for ins in blk.get("instructions", []):
                si = ins.get("sync_info")
                waits = (si or {}).get("on_wait") or []
                if si is not None and len(waits) > 1 and \
                        ins.get("opcode") not in _SEQ_OPCODES:
                    for w in waits[:-1]:
                        _wc_counter[0] += 1
                        out.append({
                            "debug": ins.get("debug", 0),
                            "engine": ins["engine"],
                            "ins": [], "outs": [],
                            "name": f"I-wc{_wc_counter[0]}",
                            "opcode": "EventSemaphore",
                            "sync_info": {"on_wait": [w], "on_update": []},
                        })
                        n_split += 1
                    si["on_wait"] = waits[-1:]
                out.append(ins)
            blk["instructions"] = out
    if n_split:
        print(f"[kernel] wait-legalizer: split {n_split} excess waits")
    return _json.dumps(d).encode()


_hook_installed = [False]


def _install_compile_hook():
    """Route every BIR->NEFF compile in this process through the wait
    legalizer (both the direct bass_utils path and the bass2jax/axon path)."""
    if _hook_installed[0]:
        return
    import concourse.bass_utils as bu

    orig = bu.compile_bir_kernel

    def patched(bir_json, tmpdir, neff_name="file.neff"):
        return orig(_legalize_bir_waits(bir_json), tmpdir, neff_name=neff_name)

    bu.compile_bir_kernel = patched
    try:
        import concourse.bass2jax as b2j
        b2j.compile_bir_kernel = patched
    except Exception:
        pass
    _hook_installed[0] = True


def _build(mask_mode, use_qkb, use_vb, phases=(1, 2, 3)):
    """Builds the Bass program. mask_mode: 'causal' | 'none' | 'general'.
    phases: debug knob to emit only a subset of the pipeline."""
    import concourse.bass as bass
    import concourse.mybir as mybir
    import concourse.tile as tile
    from concourse.masks import make_upper_triangular

    dt = mybir.dt
    f32 = dt.float32
    bf16 = dt.bfloat16
    Exp = mybir.ActivationFunctionType.Exp
    Ln = mybir.ActivationFunctionType.Ln
    Identity = mybir.ActivationFunctionType.Identity
    out_dt = bf16 if OUT_BF16 else f32

    causal = mask_mode == "causal"

    nc = bass.Bass()
    xT_d = nc.dram_tensor("xt", [DIM, T], bf16, kind="ExternalInput")
    wqT_d = nc.dram_tensor("wqt", [DIM, HSL], bf16, kind="ExternalInput")
    wkT_d = nc.dram_tensor("wkt", [DIM, HSL], bf16, kind="ExternalInput")
    wvT_d = nc.dram_tensor("wvt", [DIM, HSL], bf16, kind="ExternalInput")
    woT_d = nc.dram_tensor("wot", [HSL, DIM], bf16, kind="ExternalInput")
    outT_d = nc.dram_tensor("outT", [DIM, T], out_dt, kind="ExternalOutput")
    qb_d = kb_d = vb_d = maskT_d = None
    if use_qkb:
        qb_d = nc.dram_tensor("qb", [HSL], f32, kind="ExternalInput")
        kb_d = nc.dram_tensor("kb", [HSL], f32, kind="ExternalInput")
    if use_vb:
        vb_d = nc.dram_tensor("vb", [HSL], f32, kind="ExternalInput")
    if mask_mode == "general":
        maskT_d = nc.dram_tensor("maskt", [SEQ, SEQ], f32, kind="ExternalInput")

    # 3-D views with 128-partition-major layout
    xT_ap = xT_d[:].rearrange("(kt p) t -> p kt t", p=128)
    wq_ap = wqT_d[:].rearrange("(kt p) m -> p kt m", p=128)
    wk_ap = wkT_d[:].rearrange("(kt p) m -> p kt m", p=128)
    wv_ap = wvT_d[:].rearrange("(kt p) m -> p kt m", p=128)
    wo_ap = woT_d[:].rearrange("(g p) n -> p g n", p=128)
    outT_ap = outT_d[:].rearrange("(ot p) t -> p ot t", p=128)

    with tile.TileContext(nc) as tc:
        with (
            tc.tile_pool(name="singles", bufs=1) as singles,
            tc.tile_pool(name="xload", bufs=3) as xload,
            tc.tile_pool(name="work", bufs=4) as work,
            tc.tile_pool(name="outp", bufs=4) as outp,
            tc.tile_pool(name="psum", bufs=2, space="PSUM") as psum,
            tc.tile_pool(name="otps", bufs=4, space="PSUM") as otps,
        ):
            # ---- resident tensors -------------------------------------
            wq_sb = singles.tile([128, NKT, HSL], bf16)
            wk_sb = singles.tile([128, NKT, HSL], bf16)
            wv_sb = singles.tile([128, NKT, HSL], bf16)
            wo_sb = singles.tile([128, 2, DIM], bf16)
            for q in range(4):
                ksl = slice(4 * q, 4 * q + 4)
                nc.sync.dma_start(out=wq_sb[:, ksl], in_=wq_ap[:, ksl])
                nc.sync.dma_start(out=wk_sb[:, ksl], in_=wk_ap[:, ksl])
                nc.sync.dma_start(out=wv_sb[:, ksl], in_=wv_ap[:, ksl])
            nc.sync.dma_start(out=wo_sb, in_=wo_ap)

            qt_sb = singles.tile([128, 2, T], bf16)
            kt_sb = singles.tile([128, 2, T], bf16)
            ctxT_sb = singles.tile([128, 2, T], bf16)
            # V with 64 ones-columns per head: the P@V matmul then writes the
            # softmax denominator to PSUM partitions 64..127 (a free
            # cross-partition broadcast).
            vaug_sb = singles.tile([128, T // 128, HPC, 2 * HD], bf16)
            nc.vector.memset(vaug_sb, 1.0)

            qb_sb = kb_sb = vb_bc = None
            if use_qkb:
                qb_sb = singles.tile([128, 2], f32)
                kb_sb = singles.tile([128, 2], f32)
                nc.sync.dma_start(out=qb_sb, in_=qb_d[:].rearrange("(g p) -> p g", p=128))
                nc.sync.dma_start(out=kb_sb, in_=kb_d[:].rearrange("(g p) -> p g", p=128))
            if use_vb:
                vb_bc = singles.tile([128, HSL], f32)
                nc.sync.dma_start(out=vb_bc, in_=vb_d[:].to_broadcast([128, HSL]))

            triu_sb = None
            if causal:
                triu_sb = singles.tile([128, 128], bf16)
                make_upper_triangular(nc, triu_sb, val=1.0, diag=True)

            # ---- emission units ---------------------------------------
            # QKV projections and the output projection are emitted as
            # "filler" thunks interleaved between attention j-steps, so PE
            # always has independent matmul work while ScalarE runs the
            # softmax exps of the in-flight attention streams.

            def qkv_thunks(ch):
                tsl = slice(ch * 512, (ch + 1) * 512)
                xt_box = []

                def load():
                    xt_ch = xload.tile([128, NKT, 512], bf16, tag="xt")
                    for q in range(4):
                        ksl = slice(4 * q, 4 * q + 4)
                        nc.sync.dma_start(out=xt_ch[:, ksl],
                                          in_=xT_ap[:, ksl, tsl])
                    xt_box.append(xt_ch)
                yield load

                def qk_group(w_sb, dst_sb, b_sb, g):
                    ps = psum.tile([128, 512], f32, tag="st2", name="qk_ps")
                    for k in range(NKT):
                        nc.tensor.matmul(
                            ps, lhsT=w_sb[:, k, g * 128:(g + 1) * 128],
                            rhs=xt_box[0][:, k, :],
                            start=(k == 0), stop=(k == NKT - 1))
                    if b_sb is not None:
                        nc.scalar.activation(
                            out=dst_sb[:, g, tsl], in_=ps, func=Identity,
                            bias=b_sb[:, g:g + 1], scale=1.0)
                    else:
                        nc.vector.tensor_copy(out=dst_sb[:, g, tsl], in_=ps)

                def v_group(tt):
                    tglob = ch * 4 + tt
                    vps = psum.tile([128, HSL], f32, tag="st2", name="v_ps")
                    for k in range(NKT):
                        nc.tensor.matmul(
                            vps, lhsT=xt_box[0][:, k, tt * 128:(tt + 1) * 128],
                            rhs=wv_sb[:, k, :],
                            start=(k == 0), stop=(k == NKT - 1))
                    vdst = vaug_sb[:, tglob, :, 0:HD]
                    vsrc = vps.rearrange("p (h m) -> p h m", h=HPC)
                    if vb_bc is not None:
                        nc.vector.tensor_add(
                            out=vdst, in0=vsrc,
                            in1=vb_bc.rearrange("p (h m) -> p h m", h=HPC))
                    else:
                        nc.vector.tensor_copy(out=vdst, in_=vsrc)

                import functools
                for (w_sb, dst_sb, b_sb) in ((wq_sb, qt_sb, qb_sb),
                                             (wk_sb, kt_sb, kb_sb)):
                    for g in range(2):
                        yield functools.partial(qk_group, w_sb, dst_sb, b_sb, g)
                for tt in range(4):
                    yield functools.partial(v_group, tt)

            def oproj_thunks(ch):
                import functools
                tsl = slice(ch * 512, (ch + 1) * 512)

                def o_unit(o):
                    ops = psum.tile([128, 512], f32, tag="st2", name="o_ps")
                    for g2 in range(2):
                        nc.tensor.matmul(
                            ops, lhsT=wo_sb[:, g2, o * 128:(o + 1) * 128],
                            rhs=ctxT_sb[:, g2, tsl],
                            start=(g2 == 0), stop=(g2 == 1))
                    osb = outp.tile([128, 512], out_dt, tag="out_sb")
                    if o % 2 == 0:
                        nc.vector.tensor_copy(out=osb, in_=ops)
                    else:
                        nc.scalar.copy(out=osb, in_=ops)
                    nc.sync.dma_start(out=outT_ap[:, o, tsl], in_=osb)

                for o in range(DIM // 128):
                    yield functools.partial(o_unit, o)

            def pump(filler, n=1):
                for _ in range(n):
                    t = next(filler, None)
                    if t is None:
                        return False
                    t()
                return True

            def att_region(b, c, filler):
                """Attention for one (batch, q-chunk): head-pair streams g=0,1
                interleaved per j-step; O^T matmuls lag 2 steps; filler thunks
                are spread over the j-steps with a few reserved to bridge the
                region boundary while ScalarE drains the last exps."""
                thunks = list(filler)
                reserve = thunks[-3:]
                body = thunks[:-3]
                bi = [0]
                ots = {}
                for gg in range(2):
                    ots[gg, 0] = otps.tile([128, 512], f32, tag="ot", name="otA")
                    ots[gg, 1] = otps.tile([128, 512], f32, tag="ot", name="otB")
                jmax = 4 * c + 4 if causal else NJ
                pend = []

                def flush_ot(gg, j, qo, pt2):
                    for hh in range(2):
                        nc.tensor.matmul(
                            ots[gg, hh][:, qo:512],
                            lhsT=vaug_sb[:, b * NJ + j, 2 * gg + hh, :],
                            rhs=pt2[:, 512 * hh + qo:512 * hh + 512],
                            start=(j == 0), stop=(j == jmax - 1))

                for j in range(jmax):
                    qo = max(0, j * 128 - c * 512) if causal else 0
                    ssl = slice(b * SEQ + j * 128, b * SEQ + (j + 1) * 128)
                    qsl = slice(b * SEQ + c * 512 + qo, b * SEQ + (c + 1) * 512)
                    for gg in range(2):
                        st2 = psum.tile([128, 1024], f32, tag="st2", name="st2")
                        nc.tensor.matmul(
                            st2[:, qo:512], lhsT=kt_sb[0:64, gg, ssl],
                            rhs=qt_sb[0:64, gg, qsl],
                            start=True, stop=True, tile_position=(0, 0))
                        nc.tensor.matmul(
                            st2[:, 512 + qo:1024], lhsT=kt_sb[64:128, gg, ssl],
                            rhs=qt_sb[64:128, gg, qsl],
                            start=True, stop=True, tile_position=(64, 0))
                        if maskT_d is not None:
                            mt = work.tile([128, 512], f32, tag="mt")
                            nc.sync.dma_start(
                                out=mt,
                                in_=maskT_d[j * 128:(j + 1) * 128,
                                            c * 512:(c + 1) * 512])
                            for hh in range(2):
                                sl = slice(512 * hh, 512 * hh + 512)
                                nc.vector.tensor_add(
                                    out=st2[:, sl], in0=st2[:, sl], in1=mt)
                        pt2 = work.tile([128, 1024], bf16, tag="pt", bufs=6)
                        nc.scalar.activation(
                            out=pt2.rearrange("p (two n) -> p two n", two=2)[:, :, qo:512],
                            in_=st2.rearrange("p (two n) -> p two n", two=2)[:, :, qo:512],
                            func=Exp, scale=SCALE)
                        if causal and j * 128 >= c * 512:
                            dv = pt2.rearrange("p (two n) -> p two n", two=2)[:, :, qo:qo + 128]
                            nc.vector.tensor_mul(
                                out=dv, in0=dv,
                                in1=triu_sb[:, None, :].to_broadcast([128, 2, 128]))
                        pend.append((gg, j, qo, pt2))
                        while len(pend) > 4:
                            flush_ot(*pend.pop(0))
                    want = ((j + 1) * len(body) + jmax - 1) // jmax
                    while bi[0] < min(want, len(body)):
                        body[bi[0]]()
                        bi[0] += 1
                while pend:
                    flush_ot(*pend.pop(0))
                for t in reserve:
                    t()
                # chunk end: one f32 copy frees each accumulator slot; the
                # Ln/Exp reciprocal + multiply then run from SBUF overlapped
                # with the next region.
                for gg in range(2):
                    csl = slice(b * SEQ + c * 512, b * SEQ + (c + 1) * 512)
                    for hh in range(2):
                        ot = ots[gg, hh]
                        un = work.tile([128, 512], f32, tag="unctx")
                        nc.vector.tensor_copy(out=un, in_=ot)
                        rb = work.tile([64, 512], f32, tag="rb")
                        nc.scalar.activation(out=rb, in_=un[HD:2 * HD, :],
                                             func=Ln, scale=1.0)
                        nc.scalar.activation(out=rb, in_=rb,
                                             func=Exp, scale=-1.0)
                        nc.vector.tensor_mul(
                            out=ctxT_sb[hh * 64:(hh + 1) * 64, gg, csl],
                            in0=un[0:HD, :], in1=rb)

            # ---- schedule ---------------------------------------------
            from itertools import chain

            def drain(filler):
                while pump(filler):
                    pass

            if 1 in phases and 2 in phases and 3 in phases:
                drain(iter(qkv_thunks(0)))
                regions = [
                    (0, 0, qkv_thunks(1)),
                    (0, 1, qkv_thunks(2)),
                    (0, 2, qkv_thunks(3)),
                    (0, 3, chain(qkv_thunks(4), qkv_thunks(5))),
                    (1, 0, qkv_thunks(6)),
                    (1, 1, qkv_thunks(7)),
                    (1, 2, chain(oproj_thunks(0), oproj_thunks(1),
                                 oproj_thunks(5))),
                    (1, 3, chain(oproj_thunks(2), oproj_thunks(3),
                                 oproj_thunks(4), oproj_thunks(6))),
                ]
                for b, c, filler in regions:
                    att_region(b, c, filler)
                drain(iter(oproj_thunks(7)))
            else:
                # debug path: sequential phases
                if 1 in phases:
                    for ch in range(NCH):
                        drain(iter(qkv_thunks(ch)))
                if 2 in phases:
                    for b in range(BSZ):
                        for c in range(NCK):
                            att_region(b, c, ())
                if 3 in phases:
                    for ch in range(NCH):
                        drain(iter(oproj_thunks(ch)))

    return nc


def _classify_mask(mask):
    m = np.asarray(mask, dtype=np.float32).reshape(SEQ, SEQ)
    if not np.any(m):
        return "none", None
    lower_ok = not np.any(m[np.tril_indices(SEQ)])
    upper = m[np.triu_indices(SEQ, 1)]
    if lower_ok and np.all(np.isneginf(upper)):
        return "causal", None
    return "general", np.ascontiguousarray(m.T)


def kernel(x, start_pos, freqs_cis, mask, wq_w, wq_b, wk_w, wk_b,
           wv_w, wv_b, wo_w, wo_b):
    global LAST_RESULTS
    _install_compile_hook()
    from concourse.bass_utils import run_bass_kernel_spmd

    x = np.asarray(x, dtype=np.float32)
    mask_mode, maskT = _classify_mask(mask)
    wq_b = np.asarray(wq_b, dtype=np.float32)
    wk_b = np.asarray(wk_b, dtype=np.float32)
    wv_b = np.asarray(wv_b, dtype=np.float32)
    wo_b = np.asarray(wo_b, dtype=np.float32)
    use_qkb = bool(np.any(wq_b) or np.any(wk_b))
    use_vb = bool(np.any(wv_b))

    nc = _build(mask_mode, use_qkb, use_vb)

    xT = np.ascontiguousarray(x.reshape(T, DIM).T).astype(BF16)
    wqT = np.asarray(wq_w, dtype=np.float32).T.astype(BF16)  # [D, D]
    wkT = np.asarray(wk_w, dtype=np.float32).T.astype(BF16)
    wvT = np.asarray(wv_w, dtype=np.float32).T.astype(BF16)
    wo = np.asarray(wo_w, dtype=np.float32)

    in_maps = []
    for c in range(NCORES):
        sl = slice(HSL * c, HSL * (c + 1))
        im = {
            "xt": xT,
            "wqt": np.ascontiguousarray(wqT[:, sl]),
            "wkt": np.ascontiguousarray(wkT[:, sl]),
            "wvt": np.ascontiguousarray(wvT[:, sl]),
            "wot": np.ascontiguousarray(wo[:, sl].T).astype(BF16),
        }
        if use_qkb:
            im["qb"] = np.ascontiguousarray(wq_b[sl])
            im["kb"] = np.ascontiguousarray(wk_b[sl])
        if use_vb:
            im["vb"] = np.ascontiguousarray(wv_b[sl])
        if mask_mode == "general":
            im["maskt"] = maskT
        in_maps.append(im)

    res = run_bass_kernel_spmd(nc, in_maps, core_ids=list(range(NCORES)))
    LAST_RESULTS = res

    acc = np.zeros((DIM, T), dtype=np.float32)
    for r in res.results:
        acc += np.asarray(r["outT"], dtype=np.float32)
    out = acc.T + wo_b[None, :]
    return out.reshape(BSZ, SEQ, DIM).astype(np.float32)


# revision 22
# speedup vs baseline: 1.0818x; 1.0206x over previous
"""Trainium2 Bass kernel for nn_Attention_4930622456197.

Multi-head causal attention (B=2, S=2048, D=2048, 32 heads x head_dim 64)
with QKVO projections, tensor-parallel over heads across 8 NeuronCores
(4 heads per core).

Per-core plan (all matmul inputs bf16, f32 PSUM accumulation):
  Phase 1  QKV projections from host-pretransposed x^T [D, T]:
           Q^T, K^T in [128 (=2 heads x 64 dims), group, T] layout;
           V in natural [tok, head, 65] layout with a ones column
           appended (row 64 of V_aug.T) so the P@V matmul also produces
           the softmax denominators for free.
  Phase 2  Flash-style causal attention in score-transposed layout
           S^T[s, q] (scores never touch HBM).  exp on ScalarE with the
           1/sqrt(hd) scale folded in; no max-subtraction (scores are
           O(+-8) here, exp is safe in fp32->bf16).  The diagonal
           128x128 block of each strip is masked post-exp with a
           precomputed upper-triangular 0/1 tile.  O^T accumulates in
           PSUM over k-tiles; the 64 ones-columns of V_aug broadcast the
           softmax denominator to PSUM partitions 64..127, and 1/den is
           computed as exp(-ln(den)) on ScalarE.
  Phase 3  Row-parallel output projection producing a partial
           out^T [D, T]; host sums the 8 partials, adds wo_b.

  Emission interleaves the three phases: QKV chunks and output-projection
  tiles are "filler" thunks pumped between attention j-steps, keeping the
  PE array dense (HAM stays at 2.4 GHz) while ScalarE runs the softmax
  exps of the two in-flight head-pair streams.

The harness calls kernel(**inputs) with the full (unsharded) inputs and
expects the full [2, 2048, 2048] float32 output.
"""

import numpy as np
import ml_dtypes

BSZ, SEQ, DIM, NH = 2, 2048, 2048, 32
HD = DIM // NH            # 64
NCORES = 8
HPC = NH // NCORES        # 4 heads per core
HSL = HPC * HD            # 256 head-dims per core
T = BSZ * SEQ             # 4096 flattened tokens
SCALE = 1.0 / float(np.sqrt(HD))
BF16 = ml_dtypes.bfloat16

NKT = DIM // 128          # 16 contraction tiles over model dim
NCH = T // 512            # 8 token chunks of 512
NJ = SEQ // 128           # 16 k-tiles per sequence
NCK = SEQ // 512          # 4 q-chunks per sequence

# Output partial dtype: float32 is safest for the cross-core sum;
# bfloat16 halves the output DMA traffic.
OUT_BF16 = True

LAST_RESULTS = None       # BassKernelResults of the most recent run (for test.py)


# This walrus build caps EVERY instruction (HW-decoded and sequencer alike)
# at one sync-wait, so the legalizer splits excess waits regardless of opcode.
_SEQ_OPCODES = set()
_wc_counter = [0]


def _legalize_bir_waits(bir_bytes):
    """This container's walrus accepts only ONE sync-wait on HW-decoded
    instruction structs ("Too many sync wait commands" otherwise), but Tile
    freely emits 2-3 waits per instruction.  Split excess waits into
    standalone same-engine EventSemaphore instructions placed immediately
    before the instruction — the sequencer executes them in order, so the
    dependency semantics are identical."""
    import json as _json

    d = _json.loads(bir_bytes)
    n_split = 0
    for f in d.get("functions", []):
        for blk in f.get("blocks", []):
            out = []
            for ins in blk.get("instructions", []):
                si = ins.get("sync_info")
                waits = (si or {}).get("on_wait") or []
                if si is not None and len(waits) > 1 and \
                        ins.get("opcode") not in _SEQ_OPCODES:
                    for w in waits[:-1]:
                        _wc_counter[0] += 1
                        out.append({
                            "debug": ins.get("debug", 0),
                            "engine": ins["engine"],
                            "ins": [], "outs": [],
                            "name": f"I-wc{_wc_counter[0]}",
                            "opcode": "EventSemaphore",
                            "sync_info": {"on_wait": [w], "on_update": []},
                        })
                        n_split += 1
                    si["on_wait"] = waits[-1:]
                out.append(ins)
            blk["instructions"] = out
    if n_split:
        print(f"[kernel] wait-legalizer: split {n_split} excess waits")
    return _json.dumps(d).encode()


_hook_installed = [False]


def _install_compile_hook():
    """Route every BIR->NEFF compile in this process through the wait
    legalizer (both the direct bass_utils path and the bass2jax/axon path)."""
    if _hook_installed[0]:
        return
    import concourse.bass_utils as bu

    orig = bu.compile_bir_kernel

    def patched(bir_json, tmpdir, neff_name="file.neff"):
        return orig(_legalize_bir_waits(bir_json), tmpdir, neff_name=neff_name)

    bu.compile_bir_kernel = patched
    try:
        import concourse.bass2jax as b2j
        b2j.compile_bir_kernel = patched
    except Exception:
        pass
    _hook_installed[0] = True


def _build(mask_mode, use_qkb, use_vb, phases=(1, 2, 3)):
    """Builds the Bass program. mask_mode: 'causal' | 'none' | 'general'.
    phases: debug knob to emit only a subset of the pipeline."""
    import concourse.bass as bass
    import concourse.mybir as mybir
    import concourse.tile as tile
    from concourse.masks import make_upper_triangular

    dt = mybir.dt
    f32 = dt.float32
    bf16 = dt.bfloat16
    Exp = mybir.ActivationFunctionType.Exp
    Ln = mybir.ActivationFunctionType.Ln
    Identity = mybir.ActivationFunctionType.Identity
    out_dt = bf16 if OUT_BF16 else f32

    causal = mask_mode == "causal"

    nc = bass.Bass()
    xT_d = nc.dram_tensor("xt", [DIM, T], bf16, kind="ExternalInput")
    wqT_d = nc.dram_tensor("wqt", [DIM, HSL], bf16, kind="ExternalInput")
    wkT_d = nc.dram_tensor("wkt", [DIM, HSL], bf16, kind="ExternalInput")
    wvT_d = nc.dram_tensor("wvt", [DIM, HSL], bf16, kind="ExternalInput")
    woT_d = nc.dram_tensor("wot", [HSL, DIM], bf16, kind="ExternalInput")
    outT_d = nc.dram_tensor("outT", [DIM, T], out_dt, kind="ExternalOutput")
    qb_d = kb_d = vb_d = maskT_d = None
    if use_qkb:
        qb_d = nc.dram_tensor("qb", [HSL], f32, kind="ExternalInput")
        kb_d = nc.dram_tensor("kb", [HSL], f32, kind="ExternalInput")
    if use_vb:
        vb_d = nc.dram_tensor("vb", [HSL], f32, kind="ExternalInput")
    if mask_mode == "general":
        maskT_d = nc.dram_tensor("maskt", [SEQ, SEQ], f32, kind="ExternalInput")

    # 3-D views with 128-partition-major layout
    xT_ap = xT_d[:].rearrange("(kt p) t -> p kt t", p=128)
    wq_ap = wqT_d[:].rearrange("(kt p) m -> p kt m", p=128)
    wk_ap = wkT_d[:].rearrange("(kt p) m -> p kt m", p=128)
    wv_ap = wvT_d[:].rearrange("(kt p) m -> p kt m", p=128)
    wo_ap = woT_d[:].rearrange("(g p) n -> p g n", p=128)
    outT_ap = outT_d[:].rearrange("(ot p) t -> p ot t", p=128)

    with tile.TileContext(nc) as tc:
        with (
            tc.tile_pool(name="singles", bufs=1) as singles,
            tc.tile_pool(name="xload", bufs=3) as xload,
            tc.tile_pool(name="work", bufs=4) as work,
            tc.tile_pool(name="outp", bufs=4) as outp,
            tc.tile_pool(name="psum", bufs=2, space="PSUM") as psum,
            tc.tile_pool(name="otps", bufs=4, space="PSUM") as otps,
        ):
            # ---- resident tensors -------------------------------------
            wq_sb = singles.tile([128, NKT, HSL], bf16)
            wk_sb = singles.tile([128, NKT, HSL], bf16)
            wv_sb = singles.tile([128, NKT, HSL], bf16)
            wo_sb = singles.tile([128, 2, DIM], bf16)
            # wq is issued first so the very first Q-projection matmul can
            # start as early as possible; wk/wv/wo are issued from inside
            # qkv_thunks(0) right after the first x-chunk quarters.
            for q in range(4):
                ksl = slice(4 * q, 4 * q + 4)
                nc.sync.dma_start(out=wq_sb[:, ksl], in_=wq_ap[:, ksl])

            qt_sb = singles.tile([128, 2, T], bf16)
            kt_sb = singles.tile([128, 2, T], bf16)
            ctxT_sb = singles.tile([128, 2, T], bf16)
            # V with 64 ones-columns per head: the P@V matmul then writes the
            # softmax denominator to PSUM partitions 64..127 (a free
            # cross-partition broadcast).
            vaug_sb = singles.tile([128, T // 128, HPC, 2 * HD], bf16)
            nc.vector.memset(vaug_sb, 1.0)

            qb_sb = kb_sb = vb_bc = None
            if use_qkb:
                qb_sb = singles.tile([128, 2], f32)
                kb_sb = singles.tile([128, 2], f32)
                nc.sync.dma_start(out=qb_sb, in_=qb_d[:].rearrange("(g p) -> p g", p=128))
                nc.sync.dma_start(out=kb_sb, in_=kb_d[:].rearrange("(g p) -> p g", p=128))
            if use_vb:
                vb_bc = singles.tile([128, HSL], f32)
                nc.sync.dma_start(out=vb_bc, in_=vb_d[:].to_broadcast([128, HSL]))

            triu_sb = None
            if causal:
                triu_sb = singles.tile([128, 128], bf16)
                make_upper_triangular(nc, triu_sb, val=1.0, diag=True)

            # ---- emission units ---------------------------------------
            # QKV projections and the output projection are emitted as
            # "filler" thunks interleaved between attention j-steps, so PE
            # always has independent matmul work while ScalarE runs the
            # softmax exps of the in-flight attention streams.

            def qkv_thunks(ch):
                tsl = slice(ch * 512, (ch + 1) * 512)
                xt_box = []

                def load():
                    xt_ch = xload.tile([128, NKT, 512], bf16, tag="xt")
                    for q in range(4):
                        ksl = slice(4 * q, 4 * q + 4)
                        nc.sync.dma_start(out=xt_ch[:, ksl],
                                          in_=xT_ap[:, ksl, tsl])
                    xt_box.append(xt_ch)
                    if ch == 0:
                        for q in range(4):
                            ksl = slice(4 * q, 4 * q + 4)
                            nc.sync.dma_start(out=wk_sb[:, ksl],
                                              in_=wk_ap[:, ksl])
                            nc.sync.dma_start(out=wv_sb[:, ksl],
                                              in_=wv_ap[:, ksl])
                        nc.sync.dma_start(out=wo_sb, in_=wo_ap)
                yield load

                def qk_group(w_sb, dst_sb, b_sb, g):
                    ps = psum.tile([128, 512], f32, tag="st2", name="qk_ps")
                    for k in range(NKT):
                        nc.tensor.matmul(
                            ps, lhsT=w_sb[:, k, g * 128:(g + 1) * 128],
                            rhs=xt_box[0][:, k, :],
                            start=(k == 0), stop=(k == NKT - 1))
                    if b_sb is not None:
                        nc.scalar.activation(
                            out=dst_sb[:, g, tsl], in_=ps, func=Identity,
                            bias=b_sb[:, g:g + 1], scale=1.0)
                    else:
                        nc.vector.tensor_copy(out=dst_sb[:, g, tsl], in_=ps)

                def v_group(tt):
                    tglob = ch * 4 + tt
                    vps = psum.tile([128, HSL], f32, tag="st2", name="v_ps")
                    for k in range(NKT):
                        nc.tensor.matmul(
                            vps, lhsT=xt_box[0][:, k, tt * 128:(tt + 1) * 128],
                            rhs=wv_sb[:, k, :],
                            start=(k == 0), stop=(k == NKT - 1))
                    vdst = vaug_sb[:, tglob, :, 0:HD]
                    vsrc = vps.rearrange("p (h m) -> p h m", h=HPC)
                    if vb_bc is not None:
                        nc.vector.tensor_add(
                            out=vdst, in0=vsrc,
                            in1=vb_bc.rearrange("p (h m) -> p h m", h=HPC))
                    else:
                        nc.vector.tensor_copy(out=vdst, in_=vsrc)

                import functools
                for (w_sb, dst_sb, b_sb) in ((wq_sb, qt_sb, qb_sb),
                                             (wk_sb, kt_sb, kb_sb)):
                    for g in range(2):
                        yield functools.partial(qk_group, w_sb, dst_sb, b_sb, g)
                for tt in range(4):
                    yield functools.partial(v_group, tt)

            def oproj_thunks(ch):
                import functools
                tsl = slice(ch * 512, (ch + 1) * 512)

                def o_unit(o):
                    ops = psum.tile([128, 512], f32, tag="st2", name="o_ps")
                    for g2 in range(2):
                        nc.tensor.matmul(
                            ops, lhsT=wo_sb[:, g2, o * 128:(o + 1) * 128],
                            rhs=ctxT_sb[:, g2, tsl],
                            start=(g2 == 0), stop=(g2 == 1))
                    osb = outp.tile([128, 512], out_dt, tag="out_sb")
                    if o % 2 == 0:
                        nc.vector.tensor_copy(out=osb, in_=ops)
                    else:
                        nc.scalar.copy(out=osb, in_=ops)
                    nc.sync.dma_start(out=outT_ap[:, o, tsl], in_=osb)

                for o in range(DIM // 128):
                    yield functools.partial(o_unit, o)

            def pump(filler, n=1):
                for _ in range(n):
                    t = next(filler, None)
                    if t is None:
                        return False
                    t()
                return True

            def att_region(b, c, filler):
                """Attention for one (batch, q-chunk): head-pair streams g=0,1
                interleaved per j-step; O^T matmuls lag 2 steps; filler thunks
                are spread over the j-steps with a few reserved to bridge the
                region boundary while ScalarE drains the last exps."""
                thunks = list(filler)
                reserve = thunks[-3:]
                body = thunks[:-3]
                bi = [0]
                ots = {}
                for gg in range(2):
                    ots[gg, 0] = otps.tile([128, 512], f32, tag="ot", name="otA")
                    ots[gg, 1] = otps.tile([128, 512], f32, tag="ot", name="otB")
                jmax = 4 * c + 4 if causal else NJ
                pend = []

                def flush_ot(gg, j, qo, pt2):
                    for hh in range(2):
                        nc.tensor.matmul(
                            ots[gg, hh][:, qo:512],
                            lhsT=vaug_sb[:, b * NJ + j, 2 * gg + hh, :],
                            rhs=pt2[:, 512 * hh + qo:512 * hh + 512],
                            start=(j == 0), stop=(j == jmax - 1))

                for j in range(jmax):
                    qo = max(0, j * 128 - c * 512) if causal else 0
                    ssl = slice(b * SEQ + j * 128, b * SEQ + (j + 1) * 128)
                    qsl = slice(b * SEQ + c * 512 + qo, b * SEQ + (c + 1) * 512)
                    for gg in range(2):
                        st2 = psum.tile([128, 1024], f32, tag="st2", name="st2")
                        nc.tensor.matmul(
                            st2[:, qo:512], lhsT=kt_sb[0:64, gg, ssl],
                            rhs=qt_sb[0:64, gg, qsl],
                            start=True, stop=True, tile_position=(0, 0))
                        nc.tensor.matmul(
                            st2[:, 512 + qo:1024], lhsT=kt_sb[64:128, gg, ssl],
                            rhs=qt_sb[64:128, gg, qsl],
                            start=True, stop=True, tile_position=(64, 0))
                        if maskT_d is not None:
                            mt = work.tile([128, 512], f32, tag="mt")
                            nc.sync.dma_start(
                                out=mt,
                                in_=maskT_d[j * 128:(j + 1) * 128,
                                            c * 512:(c + 1) * 512])
                            for hh in range(2):
                                sl = slice(512 * hh, 512 * hh + 512)
                                nc.vector.tensor_add(
                                    out=st2[:, sl], in0=st2[:, sl], in1=mt)
                        pt2 = work.tile([128, 1024], bf16, tag="pt", bufs=6)
                        nc.scalar.activation(
                            out=pt2.rearrange("p (two n) -> p two n", two=2)[:, :, qo:512],
                            in_=st2.rearrange("p (two n) -> p two n", two=2)[:, :, qo:512],
                            func=Exp, scale=SCALE)
                        if causal and j * 128 >= c * 512:
                            dv = pt2.rearrange("p (two n) -> p two n", two=2)[:, :, qo:qo + 128]
                            nc.vector.tensor_mul(
                                out=dv, in0=dv,
                                in1=triu_sb[:, None, :].to_broadcast([128, 2, 128]))
                        pend.append((gg, j, qo, pt2))
                        while len(pend) > 4:
                            flush_ot(*pend.pop(0))
                    want = ((j + 1) * len(body) + jmax - 1) // jmax
                    while bi[0] < min(want, len(body)):
                        body[bi[0]]()
                        bi[0] += 1
                while pend:
                    flush_ot(*pend.pop(0))
                for t in reserve:
                    t()
                # chunk end: one f32 copy frees each accumulator slot; the
                # Ln/Exp reciprocal + multiply then run from SBUF overlapped
                # with the next region.
                for gg in range(2):
                    csl = slice(b * SEQ + c * 512, b * SEQ + (c + 1) * 512)
                    for hh in range(2):
                        ot = ots[gg, hh]
                        un = work.tile([128, 512], f32, tag="unctx")
                        nc.vector.tensor_copy(out=un, in_=ot)
                        rb = work.tile([64, 512], f32, tag="rb")
                        nc.scalar.activation(out=rb, in_=un[HD:2 * HD, :],
                                             func=Ln, scale=1.0)
                        nc.scalar.activation(out=rb, in_=rb,
                                             func=Exp, scale=-1.0)
                        nc.vector.tensor_mul(
                            out=ctxT_sb[hh * 64:(hh + 1) * 64, gg, csl],
                            in0=un[0:HD, :], in1=rb)

            # ---- schedule ---------------------------------------------
            from itertools import chain

            def drain(filler):
                while pump(filler):
                    pass

            if 1 in phases and 2 in phases and 3 in phases:
                drain(iter(qkv_thunks(0)))
                regions = [
                    (0, 0, qkv_thunks(1)),
                    (0, 1, qkv_thunks(2)),
                    (0, 2, qkv_thunks(3)),
                    (0, 3, chain(qkv_thunks(4), qkv_thunks(5))),
                    (1, 0, qkv_thunks(6)),
                    (1, 1, qkv_thunks(7)),
                    (1, 2, chain(oproj_thunks(0), oproj_thunks(1),
                                 oproj_thunks(5))),
                    (1, 3, chain(oproj_thunks(2), oproj_thunks(3),
                                 oproj_thunks(4), oproj_thunks(6))),
                ]
                for b, c, filler in regions:
                    att_region(b, c, filler)
                drain(iter(oproj_thunks(7)))
            else:
                # debug path: sequential phases
                if 1 in phases:
                    for ch in range(NCH):
                        drain(iter(qkv_thunks(ch)))
                if 2 in phases:
                    for b in range(BSZ):
                        for c in range(NCK):
                            att_region(b, c, ())
                if 3 in phases:
                    for ch in range(NCH):
                        drain(iter(oproj_thunks(ch)))

    return nc


def _classify_mask(mask):
    m = np.asarray(mask, dtype=np.float32).reshape(SEQ, SEQ)
    if not np.any(m):
        return "none", None
    lower_ok = not np.any(m[np.tril_indices(SEQ)])
    upper = m[np.triu_indices(SEQ, 1)]
    if lower_ok and np.all(np.isneginf(upper)):
        return "causal", None
    return "general", np.ascontiguousarray(m.T)


def kernel(x, start_pos, freqs_cis, mask, wq_w, wq_b, wk_w, wk_b,
           wv_w, wv_b, wo_w, wo_b):
    global LAST_RESULTS
    _install_compile_hook()
    from concourse.bass_utils import run_bass_kernel_spmd

    x = np.asarray(x, dtype=np.float32)
    mask_mode, maskT = _classify_mask(mask)
    wq_b = np.asarray(wq_b, dtype=np.float32)
    wk_b = np.asarray(wk_b, dtype=np.float32)
    wv_b = np.asarray(wv_b, dtype=np.float32)
    wo_b = np.asarray(wo_b, dtype=np.float32)
    use_qkb = bool(np.any(wq_b) or np.any(wk_b))
    use_vb = bool(np.any(wv_b))

    nc = _build(mask_mode, use_qkb, use_vb)

    xT = np.ascontiguousarray(x.reshape(T, DIM).T).astype(BF16)
    wqT = np.asarray(wq_w, dtype=np.float32).T.astype(BF16)  # [D, D]
    wkT = np.asarray(wk_w, dtype=np.float32).T.astype(BF16)
    wvT = np.asarray(wv_w, dtype=np.float32).T.astype(BF16)
    wo = np.asarray(wo_w, dtype=np.float32)

    in_maps = []
    for c in range(NCORES):
        sl = slice(HSL * c, HSL * (c + 1))
        im = {
            "xt": xT,
            "wqt": np.ascontiguousarray(wqT[:, sl]),
            "wkt": np.ascontiguousarray(wkT[:, sl]),
            "wvt": np.ascontiguousarray(wvT[:, sl]),
            "wot": np.ascontiguousarray(wo[:, sl].T).astype(BF16),
        }
        if use_qkb:
            im["qb"] = np.ascontiguousarray(wq_b[sl])
            im["kb"] = np.ascontiguousarray(wk_b[sl])
        if use_vb:
            im["vb"] = np.ascontiguousarray(wv_b[sl])
        if mask_mode == "general":
            im["maskt"] = maskT
        in_maps.append(im)

    res = run_bass_kernel_spmd(nc, in_maps, core_ids=list(range(NCORES)))
    LAST_RESULTS = res

    acc = np.zeros((DIM, T), dtype=np.float32)
    for r in res.results:
        acc += np.asarray(r["outT"], dtype=np.float32)
    out = acc.T + wo_b[None, :]
    return out.reshape(BSZ, SEQ, DIM).astype(np.float32)


# revision 23
# speedup vs baseline: 1.0871x; 1.0049x over previous
"""Trainium2 Bass kernel for nn_Attention_4930622456197.

Multi-head causal attention (B=2, S=2048, D=2048, 32 heads x head_dim 64)
with QKVO projections, tensor-parallel over heads across 8 NeuronCores
(4 heads per core).

Per-core plan (all matmul inputs bf16, f32 PSUM accumulation):
  Phase 1  QKV projections from host-pretransposed x^T [D, T]:
           Q^T, K^T in [128 (=2 heads x 64 dims), group, T] layout;
           V in natural [tok, head, 65] layout with a ones column
           appended (row 64 of V_aug.T) so the P@V matmul also produces
           the softmax denominators for free.
  Phase 2  Flash-style causal attention in score-transposed layout
           S^T[s, q] (scores never touch HBM).  exp on ScalarE with the
           1/sqrt(hd) scale folded in; no max-subtraction (scores are
           O(+-8) here, exp is safe in fp32->bf16).  The diagonal
           128x128 block of each strip is masked post-exp with a
           precomputed upper-triangular 0/1 tile.  O^T accumulates in
           PSUM over k-tiles; the 64 ones-columns of V_aug broadcast the
           softmax denominator to PSUM partitions 64..127, and 1/den is
           computed as exp(-ln(den)) on ScalarE.
  Phase 3  Row-parallel output projection producing a partial
           out^T [D, T]; host sums the 8 partials, adds wo_b.

  Emission interleaves the three phases: QKV chunks and output-projection
  tiles are "filler" thunks pumped between attention j-steps, keeping the
  PE array dense (HAM stays at 2.4 GHz) while ScalarE runs the softmax
  exps of the two in-flight head-pair streams.

The harness calls kernel(**inputs) with the full (unsharded) inputs and
expects the full [2, 2048, 2048] float32 output.
"""

import numpy as np
import ml_dtypes

BSZ, SEQ, DIM, NH = 2, 2048, 2048, 32
HD = DIM // NH            # 64
NCORES = 8
HPC = NH // NCORES        # 4 heads per core
HSL = HPC * HD            # 256 head-dims per core
T = BSZ * SEQ             # 4096 flattened tokens
SCALE = 1.0 / float(np.sqrt(HD))
BF16 = ml_dtypes.bfloat16

NKT = DIM // 128          # 16 contraction tiles over model dim
NCH = T // 512            # 8 token chunks of 512
NJ = SEQ // 128           # 16 k-tiles per sequence
NCK = SEQ // 512          # 4 q-chunks per sequence

# Output partial dtype: float32 is safest for the cross-core sum;
# bfloat16 halves the output DMA traffic.
OUT_BF16 = True

LAST_RESULTS = None       # BassKernelResults of the most recent run (for test.py)


# This walrus build caps EVERY instruction (HW-decoded and sequencer alike)
# at one sync-wait, so the legalizer splits excess waits regardless of opcode.
_SEQ_OPCODES = set()
_wc_counter = [0]


def _legalize_bir_waits(bir_bytes):
    """This container's walrus accepts only ONE sync-wait on HW-decoded
    instruction structs ("Too many sync wait commands" otherwise), but Tile
    freely emits 2-3 waits per instruction.  Split excess waits into
    standalone same-engine EventSemaphore instructions placed immediately
    before the instruction — the sequencer executes them in order, so the
    dependency semantics are identical."""
    import json as _json

    d = _json.loads(bir_bytes)
    n_split = 0
    for f in d.get("functions", []):
        for blk in f.get("blocks", []):
            out = []
            for ins in blk.get("instructions", []):
                si = ins.get("sync_info")
                waits = (si or {}).get("on_wait") or []
                if si is not None and len(waits) > 1 and \
                        ins.get("opcode") not in _SEQ_OPCODES:
                    for w in waits[:-1]:
                        _wc_counter[0] += 1
                        out.append({
                            "debug": ins.get("debug", 0),
                            "engine": ins["engine"],
                            "ins": [], "outs": [],
                            "name": f"I-wc{_wc_counter[0]}",
                            "opcode": "EventSemaphore",
                            "sync_info": {"on_wait": [w], "on_update": []},
                        })
                        n_split += 1
                    si["on_wait"] = waits[-1:]
                out.append(ins)
            blk["instructions"] = out
    if n_split:
        print(f"[kernel] wait-legalizer: split {n_split} excess waits")
    return _json.dumps(d).encode()


_hook_installed = [False]


def _install_compile_hook():
    """Route every BIR->NEFF compile in this process through the wait
    legalizer (both the direct bass_utils path and the bass2jax/axon path)."""
    if _hook_installed[0]:
        return
    import concourse.bass_utils as bu

    orig = bu.compile_bir_kernel

    def patched(bir_json, tmpdir, neff_name="file.neff"):
        return orig(_legalize_bir_waits(bir_json), tmpdir, neff_name=neff_name)

    bu.compile_bir_kernel = patched
    try:
        import concourse.bass2jax as b2j
        b2j.compile_bir_kernel = patched
    except Exception:
        pass
    _hook_installed[0] = True


def _build(mask_mode, use_qkb, use_vb, phases=(1, 2, 3)):
    """Builds the Bass program. mask_mode: 'causal' | 'none' | 'general'.
    phases: debug knob to emit only a subset of the pipeline."""
    import concourse.bass as bass
    import concourse.mybir as mybir
    import concourse.tile as tile
    from concourse.masks import make_upper_triangular

    dt = mybir.dt
    f32 = dt.float32
    bf16 = dt.bfloat16
    Exp = mybir.ActivationFunctionType.Exp
    Ln = mybir.ActivationFunctionType.Ln
    Identity = mybir.ActivationFunctionType.Identity
    out_dt = bf16 if OUT_BF16 else f32

    causal = mask_mode == "causal"

    nc = bass.Bass()
    xT_d = nc.dram_tensor("xt", [DIM, T], bf16, kind="ExternalInput")
    wqT_d = nc.dram_tensor("wqt", [DIM, HSL], bf16, kind="ExternalInput")
    wkT_d = nc.dram_tensor("wkt", [DIM, HSL], bf16, kind="ExternalInput")
    wvT_d = nc.dram_tensor("wvt", [DIM, HSL], bf16, kind="ExternalInput")
    woT_d = nc.dram_tensor("wot", [HSL, DIM], bf16, kind="ExternalInput")
    outT_d = nc.dram_tensor("outT", [DIM, T], out_dt, kind="ExternalOutput")
    qb_d = kb_d = vb_d = maskT_d = None
    if use_qkb:
        qb_d = nc.dram_tensor("qb", [HSL], f32, kind="ExternalInput")
        kb_d = nc.dram_tensor("kb", [HSL], f32, kind="ExternalInput")
    if use_vb:
        vb_d = nc.dram_tensor("vb", [HSL], f32, kind="ExternalInput")
    if mask_mode == "general":
        maskT_d = nc.dram_tensor("maskt", [SEQ, SEQ], f32, kind="ExternalInput")

    # 3-D views with 128-partition-major layout
    xT_ap = xT_d[:].rearrange("(kt p) t -> p kt t", p=128)
    wq_ap = wqT_d[:].rearrange("(kt p) m -> p kt m", p=128)
    wk_ap = wkT_d[:].rearrange("(kt p) m -> p kt m", p=128)
    wv_ap = wvT_d[:].rearrange("(kt p) m -> p kt m", p=128)
    wo_ap = woT_d[:].rearrange("(g p) n -> p g n", p=128)
    outT_ap = outT_d[:].rearrange("(ot p) t -> p ot t", p=128)

    with tile.TileContext(nc) as tc:
        with (
            tc.tile_pool(name="singles", bufs=1) as singles,
            tc.tile_pool(name="xload", bufs=3) as xload,
            tc.tile_pool(name="work", bufs=4) as work,
            tc.tile_pool(name="outp", bufs=4) as outp,
            tc.tile_pool(name="psum", bufs=2, space="PSUM") as psum,
            tc.tile_pool(name="otps", bufs=4, space="PSUM") as otps,
        ):
            # ---- resident tensors -------------------------------------
            wq_sb = singles.tile([128, NKT, HSL], bf16)
            wk_sb = singles.tile([128, NKT, HSL], bf16)
            wv_sb = singles.tile([128, NKT, HSL], bf16)
            wo_sb = singles.tile([128, 2, DIM], bf16)
            # wq is issued first so the very first Q-projection matmul can
            # start as early as possible; wk/wv/wo are issued from inside
            # qkv_thunks(0) right after the first x-chunk quarters.
            for q in range(4):
                ksl = slice(4 * q, 4 * q + 4)
                nc.sync.dma_start(out=wq_sb[:, ksl], in_=wq_ap[:, ksl])

            qt_sb = singles.tile([128, 2, T], bf16)
            kt_sb = singles.tile([128, 2, T], bf16)
            ctxT_sb = singles.tile([128, 2, T], bf16)
            # V with 64 ones-columns per head: the P@V matmul then writes the
            # softmax denominator to PSUM partitions 64..127 (a free
            # cross-partition broadcast).
            vaug_sb = singles.tile([128, T // 128, HPC, 2 * HD], bf16)
            nc.vector.memset(vaug_sb, 1.0)

            qb_sb = kb_sb = vb_bc = None
            if use_qkb:
                qb_sb = singles.tile([128, 2], f32)
                kb_sb = singles.tile([128, 2], f32)
                nc.sync.dma_start(out=qb_sb, in_=qb_d[:].rearrange("(g p) -> p g", p=128))
                nc.sync.dma_start(out=kb_sb, in_=kb_d[:].rearrange("(g p) -> p g", p=128))
            if use_vb:
                vb_bc = singles.tile([128, HSL], f32)
                nc.sync.dma_start(out=vb_bc, in_=vb_d[:].to_broadcast([128, HSL]))

            triu_sb = None
            if causal:
                triu_sb = singles.tile([128, 128], bf16)
                make_upper_triangular(nc, triu_sb, val=1.0, diag=True)

            # ---- emission units ---------------------------------------
            # QKV projections and the output projection are emitted as
            # "filler" thunks interleaved between attention j-steps, so PE
            # always has independent matmul work while ScalarE runs the
            # softmax exps of the in-flight attention streams.

            def qkv_thunks(ch):
                tsl = slice(ch * 512, (ch + 1) * 512)
                xt_box = []

                def load():
                    xt_ch = xload.tile([128, NKT, 512], bf16, tag="xt")
                    for q in range(4):
                        ksl = slice(4 * q, 4 * q + 4)
                        nc.sync.dma_start(out=xt_ch[:, ksl],
                                          in_=xT_ap[:, ksl, tsl])
                    xt_box.append(xt_ch)
                    if ch == 0:
                        for q in range(4):
                            ksl = slice(4 * q, 4 * q + 4)
                            nc.sync.dma_start(out=wk_sb[:, ksl],
                                              in_=wk_ap[:, ksl])
                            nc.sync.dma_start(out=wv_sb[:, ksl],
                                              in_=wv_ap[:, ksl])
                        nc.sync.dma_start(out=wo_sb, in_=wo_ap)
                yield load

                def qk_group(w_sb, dst_sb, b_sb, g):
                    ps = psum.tile([128, 512], f32, tag="st2", name="qk_ps")
                    for k in range(NKT):
                        nc.tensor.matmul(
                            ps, lhsT=w_sb[:, k, g * 128:(g + 1) * 128],
                            rhs=xt_box[0][:, k, :],
                            start=(k == 0), stop=(k == NKT - 1))
                    if b_sb is not None:
                        nc.scalar.activation(
                            out=dst_sb[:, g, tsl], in_=ps, func=Identity,
                            bias=b_sb[:, g:g + 1], scale=1.0)
                    else:
                        nc.vector.tensor_copy(out=dst_sb[:, g, tsl], in_=ps)

                def v_group(tt):
                    tglob = ch * 4 + tt
                    vps = psum.tile([128, HSL], f32, tag="st2", name="v_ps")
                    for k in range(NKT):
                        nc.tensor.matmul(
                            vps, lhsT=xt_box[0][:, k, tt * 128:(tt + 1) * 128],
                            rhs=wv_sb[:, k, :],
                            start=(k == 0), stop=(k == NKT - 1))
                    vdst = vaug_sb[:, tglob, :, 0:HD]
                    vsrc = vps.rearrange("p (h m) -> p h m", h=HPC)
                    if vb_bc is not None:
                        nc.vector.tensor_add(
                            out=vdst, in0=vsrc,
                            in1=vb_bc.rearrange("p (h m) -> p h m", h=HPC))
                    else:
                        nc.vector.tensor_copy(out=vdst, in_=vsrc)

                import functools
                for (w_sb, dst_sb, b_sb) in ((wq_sb, qt_sb, qb_sb),
                                             (wk_sb, kt_sb, kb_sb)):
                    for g in range(2):
                        yield functools.partial(qk_group, w_sb, dst_sb, b_sb, g)
                for tt in range(4):
                    yield functools.partial(v_group, tt)

            def oproj_thunks(ch):
                import functools
                tsl = slice(ch * 512, (ch + 1) * 512)

                def o_unit(o):
                    ops = psum.tile([128, 512], f32, tag="st2", name="o_ps")
                    for g2 in range(2):
                        nc.tensor.matmul(
                            ops, lhsT=wo_sb[:, g2, o * 128:(o + 1) * 128],
                            rhs=ctxT_sb[:, g2, tsl],
                            start=(g2 == 0), stop=(g2 == 1))
                    osb = outp.tile([128, 512], out_dt, tag="out_sb")
                    if o % 2 == 0:
                        nc.vector.tensor_copy(out=osb, in_=ops)
                    else:
                        nc.scalar.copy(out=osb, in_=ops)
                    nc.sync.dma_start(out=outT_ap[:, o, tsl], in_=osb)

                for o in range(DIM // 128):
                    yield functools.partial(o_unit, o)

            def pump(filler, n=1):
                for _ in range(n):
                    t = next(filler, None)
                    if t is None:
                        return False
                    t()
                return True

            def att_region(b, c, filler):
                """Attention for one (batch, q-chunk): head-pair streams g=0,1
                interleaved per j-step; O^T matmuls lag 2 steps; filler thunks
                are spread over the j-steps with a few reserved to bridge the
                region boundary while ScalarE drains the last exps."""
                thunks = list(filler)
                reserve = thunks[-3:]
                body = thunks[:-3]
                bi = [0]
                ots = {}
                for gg in range(2):
                    ots[gg, 0] = otps.tile([128, 512], f32, tag="ot", name="otA")
                    ots[gg, 1] = otps.tile([128, 512], f32, tag="ot", name="otB")
                jmax = 4 * c + 4 if causal else NJ
                pend = []

                def flush_ot(gg, j, qo, pt2):
                    for hh in range(2):
                        nc.tensor.matmul(
                            ots[gg, hh][:, qo:512],
                            lhsT=vaug_sb[:, b * NJ + j, 2 * gg + hh, :],
                            rhs=pt2[:, 512 * hh + qo:512 * hh + 512],
                            start=(j == 0), stop=(j == jmax - 1))

                for j in range(jmax):
                    qo = max(0, j * 128 - c * 512) if causal else 0
                    ssl = slice(b * SEQ + j * 128, b * SEQ + (j + 1) * 128)
                    qsl = slice(b * SEQ + c * 512 + qo, b * SEQ + (c + 1) * 512)
                    for gg in range(2):
                        st2 = psum.tile([128, 1024], f32, tag="st2", name="st2")
                        nc.tensor.matmul(
                            st2[:, qo:512], lhsT=kt_sb[0:64, gg, ssl],
                            rhs=qt_sb[0:64, gg, qsl],
                            start=True, stop=True, tile_position=(0, 0))
                        nc.tensor.matmul(
                            st2[:, 512 + qo:1024], lhsT=kt_sb[64:128, gg, ssl],
                            rhs=qt_sb[64:128, gg, qsl],
                            start=True, stop=True, tile_position=(64, 0))
                        if maskT_d is not None:
                            mt = work.tile([128, 512], f32, tag="mt")
                            nc.sync.dma_start(
                                out=mt,
                                in_=maskT_d[j * 128:(j + 1) * 128,
                                            c * 512:(c + 1) * 512])
                            for hh in range(2):
                                sl = slice(512 * hh, 512 * hh + 512)
                                nc.vector.tensor_add(
                                    out=st2[:, sl], in0=st2[:, sl], in1=mt)
                        pt2 = work.tile([128, 1024], bf16, tag="pt", bufs=8)
                        nc.scalar.activation(
                            out=pt2.rearrange("p (two n) -> p two n", two=2)[:, :, qo:512],
                            in_=st2.rearrange("p (two n) -> p two n", two=2)[:, :, qo:512],
                            func=Exp, scale=SCALE)
                        if causal and j * 128 >= c * 512:
                            dv = pt2.rearrange("p (two n) -> p two n", two=2)[:, :, qo:qo + 128]
                            nc.vector.tensor_mul(
                                out=dv, in0=dv,
                                in1=triu_sb[:, None, :].to_broadcast([128, 2, 128]))
                        pend.append((gg, j, qo, pt2))
                        while len(pend) > 6:
                            flush_ot(*pend.pop(0))
                    want = ((j + 1) * len(body) + jmax - 1) // jmax
                    while bi[0] < min(want, len(body)):
                        body[bi[0]]()
                        bi[0] += 1
                while pend:
                    flush_ot(*pend.pop(0))
                for t in reserve:
                    t()
                # chunk end: one f32 copy frees each accumulator slot; the
                # Ln/Exp reciprocal + multiply then run from SBUF overlapped
                # with the next region.
                for gg in range(2):
                    csl = slice(b * SEQ + c * 512, b * SEQ + (c + 1) * 512)
                    for hh in range(2):
                        ot = ots[gg, hh]
                        un = work.tile([128, 512], f32, tag="unctx")
                        nc.vector.tensor_copy(out=un, in_=ot)
                        rb = work.tile([64, 512], f32, tag="rb")
                        nc.scalar.activation(out=rb, in_=un[HD:2 * HD, :],
                                             func=Ln, scale=1.0)
                        nc.scalar.activation(out=rb, in_=rb,
                                             func=Exp, scale=-1.0)
                        nc.vector.tensor_mul(
                            out=ctxT_sb[hh * 64:(hh + 1) * 64, gg, csl],
                            in0=un[0:HD, :], in1=rb)

            # ---- schedule ---------------------------------------------
            from itertools import chain

            def drain(filler):
                while pump(filler):
                    pass

            if 1 in phases and 2 in phases and 3 in phases:
                drain(iter(qkv_thunks(0)))
                regions = [
                    (0, 0, qkv_thunks(1)),
                    (0, 1, qkv_thunks(2)),
                    (0, 2, qkv_thunks(3)),
                    (0, 3, chain(qkv_thunks(4), qkv_thunks(5))),
                    (1, 0, qkv_thunks(6)),
                    (1, 1, qkv_thunks(7)),
                    (1, 2, chain(oproj_thunks(0), oproj_thunks(1),
                                 oproj_thunks(5))),
                    (1, 3, chain(oproj_thunks(2), oproj_thunks(3),
                                 oproj_thunks(4), oproj_thunks(6))),
                ]
                for b, c, filler in regions:
                    att_region(b, c, filler)
                drain(iter(oproj_thunks(7)))
            else:
                # debug path: sequential phases
                if 1 in phases:
                    for ch in range(NCH):
                        drain(iter(qkv_thunks(ch)))
                if 2 in phases:
                    for b in range(BSZ):
                        for c in range(NCK):
                            att_region(b, c, ())
                if 3 in phases:
                    for ch in range(NCH):
                        drain(iter(oproj_thunks(ch)))

    return nc


def _classify_mask(mask):
    m = np.asarray(mask, dtype=np.float32).reshape(SEQ, SEQ)
    if not np.any(m):
        return "none", None
    lower_ok = not np.any(m[np.tril_indices(SEQ)])
    upper = m[np.triu_indices(SEQ, 1)]
    if lower_ok and np.all(np.isneginf(upper)):
        return "causal", None
    return "general", np.ascontiguousarray(m.T)


def kernel(x, start_pos, freqs_cis, mask, wq_w, wq_b, wk_w, wk_b,
           wv_w, wv_b, wo_w, wo_b):
    global LAST_RESULTS
    _install_compile_hook()
    from concourse.bass_utils import run_bass_kernel_spmd

    x = np.asarray(x, dtype=np.float32)
    mask_mode, maskT = _classify_mask(mask)
    wq_b = np.asarray(wq_b, dtype=np.float32)
    wk_b = np.asarray(wk_b, dtype=np.float32)
    wv_b = np.asarray(wv_b, dtype=np.float32)
    wo_b = np.asarray(wo_b, dtype=np.float32)
    use_qkb = bool(np.any(wq_b) or np.any(wk_b))
    use_vb = bool(np.any(wv_b))

    nc = _build(mask_mode, use_qkb, use_vb)

    xT = np.ascontiguousarray(x.reshape(T, DIM).T).astype(BF16)
    wqT = np.asarray(wq_w, dtype=np.float32).T.astype(BF16)  # [D, D]
    wkT = np.asarray(wk_w, dtype=np.float32).T.astype(BF16)
    wvT = np.asarray(wv_w, dtype=np.float32).T.astype(BF16)
    wo = np.asarray(wo_w, dtype=np.float32)

    in_maps = []
    for c in range(NCORES):
        sl = slice(HSL * c, HSL * (c + 1))
        im = {
            "xt": xT,
            "wqt": np.ascontiguousarray(wqT[:, sl]),
            "wkt": np.ascontiguousarray(wkT[:, sl]),
            "wvt": np.ascontiguousarray(wvT[:, sl]),
            "wot": np.ascontiguousarray(wo[:, sl].T).astype(BF16),
        }
        if use_qkb:
            im["qb"] = np.ascontiguousarray(wq_b[sl])
            im["kb"] = np.ascontiguousarray(wk_b[sl])
        if use_vb:
            im["vb"] = np.ascontiguousarray(wv_b[sl])
        if mask_mode == "general":
            im["maskt"] = maskT
        in_maps.append(im)

    res = run_bass_kernel_spmd(nc, in_maps, core_ids=list(range(NCORES)))
    LAST_RESULTS = res

    acc = np.zeros((DIM, T), dtype=np.float32)
    for r in res.results:
        acc += np.asarray(r["outT"], dtype=np.float32)
    out = acc.T + wo_b[None, :]
    return out.reshape(BSZ, SEQ, DIM).astype(np.float32)


# revision 24
# speedup vs baseline: 1.1316x; 1.0410x over previous
"""Trainium2 Bass kernel for nn_Attention_4930622456197.

Multi-head causal attention (B=2, S=2048, D=2048, 32 heads x head_dim 64)
with QKVO projections, tensor-parallel over heads across 8 NeuronCores
(4 heads per core).

Per-core plan (all matmul inputs bf16, f32 PSUM accumulation):
  Phase 1  QKV projections from host-pretransposed x^T [D, T]:
           Q^T, K^T in [128 (=2 heads x 64 dims), group, T] layout;
           V in natural [tok, head, 65] layout with a ones column
           appended (row 64 of V_aug.T) so the P@V matmul also produces
           the softmax denominators for free.
  Phase 2  Flash-style causal attention in score-transposed layout
           S^T[s, q] (scores never touch HBM).  exp on ScalarE with the
           1/sqrt(hd) scale folded in; no max-subtraction (scores are
           O(+-8) here, exp is safe in fp32->bf16).  The diagonal
           128x128 block of each strip is masked post-exp with a
           precomputed upper-triangular 0/1 tile.  O^T accumulates in
           PSUM over k-tiles; the 64 ones-columns of V_aug broadcast the
           softmax denominator to PSUM partitions 64..127, and 1/den is
           computed as exp(-ln(den)) on ScalarE.
  Phase 3  Row-parallel output projection producing a partial
           out^T [D, T]; host sums the 8 partials, adds wo_b.

  Emission interleaves the three phases: QKV chunks and output-projection
  tiles are "filler" thunks pumped between attention j-steps, keeping the
  PE array dense (HAM stays at 2.4 GHz) while ScalarE runs the softmax
  exps of the two in-flight head-pair streams.

The harness calls kernel(**inputs) with the full (unsharded) inputs and
expects the full [2, 2048, 2048] float32 output.
"""

import numpy as np
import ml_dtypes

BSZ, SEQ, DIM, NH = 2, 2048, 2048, 32
HD = DIM // NH            # 64
NCORES = 8
HPC = NH // NCORES        # 4 heads per core
HSL = HPC * HD            # 256 head-dims per core
T = BSZ * SEQ             # 4096 flattened tokens
SCALE = 1.0 / float(np.sqrt(HD))
BF16 = ml_dtypes.bfloat16

NKT = DIM // 128          # 16 contraction tiles over model dim
NCH = T // 512            # 8 token chunks of 512
NJ = SEQ // 128           # 16 k-tiles per sequence
NCK = SEQ // 512          # 4 q-chunks per sequence

# Output partial dtype: float32 is safest for the cross-core sum;
# bfloat16 halves the output DMA traffic.
OUT_BF16 = True

LAST_RESULTS = None       # BassKernelResults of the most recent run (for test.py)


# This walrus build caps EVERY instruction (HW-decoded and sequencer alike)
# at one sync-wait, so the legalizer splits excess waits regardless of opcode.
_SEQ_OPCODES = set()
_wc_counter = [0]


def _legalize_bir_waits(bir_bytes):
    """This container's walrus accepts only ONE sync-wait on HW-decoded
    instruction structs ("Too many sync wait commands" otherwise), but Tile
    freely emits 2-3 waits per instruction.  Split excess waits into
    standalone same-engine EventSemaphore instructions placed immediately
    before the instruction — the sequencer executes them in order, so the
    dependency semantics are identical."""
    import json as _json

    d = _json.loads(bir_bytes)
    n_split = 0
    for f in d.get("functions", []):
        for blk in f.get("blocks", []):
            out = []
            for ins in blk.get("instructions", []):
                si = ins.get("sync_info")
                waits = (si or {}).get("on_wait") or []
                if si is not None and len(waits) > 1 and \
                        ins.get("opcode") not in _SEQ_OPCODES:
                    for w in waits[:-1]:
                        _wc_counter[0] += 1
                        out.append({
                            "debug": ins.get("debug", 0),
                            "engine": ins["engine"],
                            "ins": [], "outs": [],
                            "name": f"I-wc{_wc_counter[0]}",
                            "opcode": "EventSemaphore",
                            "sync_info": {"on_wait": [w], "on_update": []},
                        })
                        n_split += 1
                    si["on_wait"] = waits[-1:]
                out.append(ins)
            blk["instructions"] = out
    if n_split:
        print(f"[kernel] wait-legalizer: split {n_split} excess waits")
    return _json.dumps(d).encode()


_hook_installed = [False]


def _install_compile_hook():
    """Route every BIR->NEFF compile in this process through the wait
    legalizer (both the direct bass_utils path and the bass2jax/axon path)."""
    if _hook_installed[0]:
        return
    import concourse.bass_utils as bu

    orig = bu.compile_bir_kernel

    def patched(bir_json, tmpdir, neff_name="file.neff"):
        return orig(_legalize_bir_waits(bir_json), tmpdir, neff_name=neff_name)

    bu.compile_bir_kernel = patched
    try:
        import concourse.bass2jax as b2j
        b2j.compile_bir_kernel = patched
    except Exception:
        pass
    _hook_installed[0] = True


def _build(mask_mode, use_qkb, use_vb, phases=(1, 2, 3)):
    """Builds the Bass program. mask_mode: 'causal' | 'none' | 'general'.
    phases: debug knob to emit only a subset of the pipeline."""
    import concourse.bass as bass
    import concourse.mybir as mybir
    import concourse.tile as tile
    from concourse.masks import make_upper_triangular

    dt = mybir.dt
    f32 = dt.float32
    bf16 = dt.bfloat16
    Exp = mybir.ActivationFunctionType.Exp
    Ln = mybir.ActivationFunctionType.Ln
    Identity = mybir.ActivationFunctionType.Identity
    out_dt = bf16 if OUT_BF16 else f32

    causal = mask_mode == "causal"

    nc = bass.Bass()
    xT_d = nc.dram_tensor("xt", [DIM, T], bf16, kind="ExternalInput")
    wqT_d = nc.dram_tensor("wqt", [DIM, HSL], bf16, kind="ExternalInput")
    wkT_d = nc.dram_tensor("wkt", [DIM, HSL], bf16, kind="ExternalInput")
    wvT_d = nc.dram_tensor("wvt", [DIM, HSL], bf16, kind="ExternalInput")
    woT_d = nc.dram_tensor("wot", [HSL, DIM], bf16, kind="ExternalInput")
    outT_d = nc.dram_tensor("outT", [DIM, T], out_dt, kind="ExternalOutput")
    qb_d = kb_d = vb_d = maskT_d = None
    if use_qkb:
        qb_d = nc.dram_tensor("qb", [HSL], f32, kind="ExternalInput")
        kb_d = nc.dram_tensor("kb", [HSL], f32, kind="ExternalInput")
    if use_vb:
        vb_d = nc.dram_tensor("vb", [HSL], f32, kind="ExternalInput")
    if mask_mode == "general":
        maskT_d = nc.dram_tensor("maskt", [SEQ, SEQ], f32, kind="ExternalInput")

    # 3-D views with 128-partition-major layout
    xT_ap = xT_d[:].rearrange("(kt p) t -> p kt t", p=128)
    wq_ap = wqT_d[:].rearrange("(kt p) m -> p kt m", p=128)
    wk_ap = wkT_d[:].rearrange("(kt p) m -> p kt m", p=128)
    wv_ap = wvT_d[:].rearrange("(kt p) m -> p kt m", p=128)
    wo_ap = woT_d[:].rearrange("(g p) n -> p g n", p=128)
    outT_ap = outT_d[:].rearrange("(ot p) t -> p ot t", p=128)

    with tile.TileContext(nc) as tc:
        with (
            tc.tile_pool(name="singles", bufs=1) as singles,
            tc.tile_pool(name="xload", bufs=3) as xload,
            tc.tile_pool(name="work", bufs=4) as work,
            tc.tile_pool(name="outp", bufs=4) as outp,
            tc.tile_pool(name="psum", bufs=2, space="PSUM") as psum,
            tc.tile_pool(name="otps", bufs=4, space="PSUM") as otps,
        ):
            # ---- resident tensors -------------------------------------
            wq_sb = singles.tile([128, NKT, HSL], bf16)
            wk_sb = singles.tile([128, NKT, HSL], bf16)
            wv_sb = singles.tile([128, NKT, HSL], bf16)
            wo_sb = singles.tile([128, 2, DIM], bf16)
            # wq is issued first so the very first Q-projection matmul can
            # start as early as possible; wk/wv/wo are issued from inside
            # qkv_thunks(0) right after the first x-chunk quarters.
            for q in range(4):
                ksl = slice(4 * q, 4 * q + 4)
                nc.sync.dma_start(out=wq_sb[:, ksl], in_=wq_ap[:, ksl])

            qt_sb = singles.tile([128, 2, T], bf16)
            kt_sb = singles.tile([128, 2, T], bf16)
            ctxT_sb = singles.tile([128, 2, T], bf16)
            # V with 64 ones-columns per head: the P@V matmul then writes the
            # softmax denominator to PSUM partitions 64..127 (a free
            # cross-partition broadcast).
            vaug_sb = singles.tile([128, T // 128, HPC, 2 * HD], bf16)
            nc.vector.memset(vaug_sb, 1.0)

            qb_sb = kb_sb = vb_bc = None
            if use_qkb:
                qb_sb = singles.tile([128, 2], f32)
                kb_sb = singles.tile([128, 2], f32)
                nc.sync.dma_start(out=qb_sb, in_=qb_d[:].rearrange("(g p) -> p g", p=128))
                nc.sync.dma_start(out=kb_sb, in_=kb_d[:].rearrange("(g p) -> p g", p=128))
            if use_vb:
                vb_bc = singles.tile([128, HSL], f32)
                nc.sync.dma_start(out=vb_bc, in_=vb_d[:].to_broadcast([128, HSL]))

            triu_sb = None
            if causal:
                triu_sb = singles.tile([128, 128], bf16)
                make_upper_triangular(nc, triu_sb, val=1.0, diag=True)

            # ---- emission units ---------------------------------------
            # QKV projections and the output projection are emitted as
            # "filler" thunks interleaved between attention j-steps, so PE
            # always has independent matmul work while ScalarE runs the
            # softmax exps of the in-flight attention streams.

            def qkv_thunks(ch):
                tsl = slice(ch * 512, (ch + 1) * 512)
                xt_box = []

                def load():
                    xt_ch = xload.tile([128, NKT, 512], bf16, tag="xt")
                    for q in range(4):
                        ksl = slice(4 * q, 4 * q + 4)
                        nc.sync.dma_start(out=xt_ch[:, ksl],
                                          in_=xT_ap[:, ksl, tsl])
                    xt_box.append(xt_ch)
                    if ch == 0:
                        for q in range(4):
                            ksl = slice(4 * q, 4 * q + 4)
                            nc.sync.dma_start(out=wk_sb[:, ksl],
                                              in_=wk_ap[:, ksl])
                            nc.sync.dma_start(out=wv_sb[:, ksl],
                                              in_=wv_ap[:, ksl])
                        nc.sync.dma_start(out=wo_sb, in_=wo_ap)
                yield load

                # both head-pair groups of one projection share a single
                # 2-bank PSUM tile (halves filler slot pressure in the st2
                # rotation) and evict with one instruction
                def qk_pair(w_sb, dst_sb, b_sb):
                    ps2 = psum.tile([128, 1024], f32, tag="st2", name="qk2")
                    for g in range(2):
                        for k in range(NKT):
                            nc.tensor.matmul(
                                ps2[:, g * 512:(g + 1) * 512],
                                lhsT=w_sb[:, k, g * 128:(g + 1) * 128],
                                rhs=xt_box[0][:, k, :],
                                start=(k == 0), stop=(k == NKT - 1))
                    if b_sb is not None:
                        for g in range(2):
                            nc.scalar.activation(
                                out=dst_sb[:, g, tsl],
                                in_=ps2[:, g * 512:(g + 1) * 512],
                                func=Identity, bias=b_sb[:, g:g + 1], scale=1.0)
                    else:
                        nc.vector.tensor_copy(
                            out=dst_sb[:, :, tsl],
                            in_=ps2.rearrange("p (g n) -> p g n", g=2))

                def v_pair(tp):
                    ps2 = psum.tile([128, 1024], f32, tag="st2", name="v2")
                    for i in range(2):
                        tt = 2 * tp + i
                        for k in range(NKT):
                            nc.tensor.matmul(
                                ps2[:, i * 512:i * 512 + HSL],
                                lhsT=xt_box[0][:, k, tt * 128:(tt + 1) * 128],
                                rhs=wv_sb[:, k, :],
                                start=(k == 0), stop=(k == NKT - 1))
                    tg0 = ch * 4 + 2 * tp
                    vdst = vaug_sb[:, tg0:tg0 + 2, :, 0:HD]
                    vsrc = ps2.rearrange("p (i n) -> p i n", i=2)[:, :, 0:HSL]
                    vsrc = vsrc.rearrange("p i (h m) -> p i h m", h=HPC)
                    if vb_bc is not None:
                        nc.vector.tensor_add(
                            out=vdst, in0=vsrc,
                            in1=vb_bc[:, None, :].to_broadcast(
                                [128, 2, HSL]).rearrange(
                                "p i (h m) -> p i h m", h=HPC))
                    else:
                        nc.vector.tensor_copy(out=vdst, in_=vsrc)

                import functools
                yield functools.partial(qk_pair, wq_sb, qt_sb, qb_sb)
                yield functools.partial(qk_pair, wk_sb, kt_sb, kb_sb)
                for tp in range(2):
                    yield functools.partial(v_pair, tp)

            def oproj_thunks(ch):
                import functools
                tsl = slice(ch * 512, (ch + 1) * 512)

                def o_pair(op):
                    ps2 = psum.tile([128, 1024], f32, tag="st2", name="o2")
                    for i in range(2):
                        o = 2 * op + i
                        for g2 in range(2):
                            nc.tensor.matmul(
                                ps2[:, i * 512:(i + 1) * 512],
                                lhsT=wo_sb[:, g2, o * 128:(o + 1) * 128],
                                rhs=ctxT_sb[:, g2, tsl],
                                start=(g2 == 0), stop=(g2 == 1))
                    osb = outp.tile([128, 2, 512], out_dt, tag="out_sb")
                    src2 = ps2.rearrange("p (i n) -> p i n", i=2)
                    if op % 2 == 0:
                        nc.vector.tensor_copy(out=osb, in_=src2)
                    else:
                        nc.scalar.copy(out=osb, in_=src2)
                    nc.sync.dma_start(
                        out=outT_ap[:, 2 * op:2 * op + 2, tsl], in_=osb)

                for op in range(DIM // 256):
                    yield functools.partial(o_pair, op)

            def pump(filler, n=1):
                for _ in range(n):
                    t = next(filler, None)
                    if t is None:
                        return False
                    t()
                return True

            def att_region(b, c, filler):
                """Attention for one (batch, q-chunk): head-pair streams g=0,1
                interleaved per j-step; O^T matmuls lag 2 steps; filler thunks
                are spread over the j-steps with a few reserved to bridge the
                region boundary while ScalarE drains the last exps."""
                thunks = list(filler)
                reserve = thunks[-3:]
                body = thunks[:-3]
                bi = [0]
                ots = {}
                for gg in range(2):
                    ots[gg, 0] = otps.tile([128, 512], f32, tag="ot", name="otA")
                    ots[gg, 1] = otps.tile([128, 512], f32, tag="ot", name="otB")
                jmax = 4 * c + 4 if causal else NJ
                pend = []

                def flush_ot(gg, j, qo, pt2):
                    for hh in range(2):
                        nc.tensor.matmul(
                            ots[gg, hh][:, qo:512],
                            lhsT=vaug_sb[:, b * NJ + j, 2 * gg + hh, :],
                            rhs=pt2[:, 512 * hh + qo:512 * hh + 512],
                            start=(j == 0), stop=(j == jmax - 1))

                for j in range(jmax):
                    qo = max(0, j * 128 - c * 512) if causal else 0
                    ssl = slice(b * SEQ + j * 128, b * SEQ + (j + 1) * 128)
                    qsl = slice(b * SEQ + c * 512 + qo, b * SEQ + (c + 1) * 512)
                    for gg in range(2):
                        st2 = psum.tile([128, 1024], f32, tag="st2", name="st2")
                        nc.tensor.matmul(
                            st2[:, qo:512], lhsT=kt_sb[0:64, gg, ssl],
                            rhs=qt_sb[0:64, gg, qsl],
                            start=True, stop=True, tile_position=(0, 0))
                        nc.tensor.matmul(
                            st2[:, 512 + qo:1024], lhsT=kt_sb[64:128, gg, ssl],
                            rhs=qt_sb[64:128, gg, qsl],
                            start=True, stop=True, tile_position=(64, 0))
                        if maskT_d is not None:
                            mt = work.tile([128, 512], f32, tag="mt")
                            nc.sync.dma_start(
                                out=mt,
                                in_=maskT_d[j * 128:(j + 1) * 128,
                                            c * 512:(c + 1) * 512])
                            for hh in range(2):
                                sl = slice(512 * hh, 512 * hh + 512)
                                nc.vector.tensor_add(
                                    out=st2[:, sl], in0=st2[:, sl], in1=mt)
                        pt2 = work.tile([128, 1024], bf16, tag="pt", bufs=8)
                        nc.scalar.activation(
                            out=pt2.rearrange("p (two n) -> p two n", two=2)[:, :, qo:512],
                            in_=st2.rearrange("p (two n) -> p two n", two=2)[:, :, qo:512],
                            func=Exp, scale=SCALE)
                        if causal and j * 128 >= c * 512:
                            dv = pt2.rearrange("p (two n) -> p two n", two=2)[:, :, qo:qo + 128]
                            nc.vector.tensor_mul(
                                out=dv, in0=dv,
                                in1=triu_sb[:, None, :].to_broadcast([128, 2, 128]))
                        pend.append((gg, j, qo, pt2))
                        while len(pend) > 6:
                            flush_ot(*pend.pop(0))
                    want = ((j + 1) * len(body) + jmax - 1) // jmax
                    while bi[0] < min(want, len(body)):
                        body[bi[0]]()
                        bi[0] += 1
                while pend:
                    flush_ot(*pend.pop(0))
                for t in reserve:
                    t()
                # chunk end: one f32 copy frees each accumulator slot; the
                # Ln/Exp reciprocal + multiply then run from SBUF overlapped
                # with the next region.
                for gg in range(2):
                    csl = slice(b * SEQ + c * 512, b * SEQ + (c + 1) * 512)
                    for hh in range(2):
                        ot = ots[gg, hh]
                        un = work.tile([128, 512], f32, tag="unctx")
                        nc.vector.tensor_copy(out=un, in_=ot)
                        rb = work.tile([64, 512], f32, tag="rb")
                        nc.scalar.activation(out=rb, in_=un[HD:2 * HD, :],
                                             func=Ln, scale=1.0)
                        nc.scalar.activation(out=rb, in_=rb,
                                             func=Exp, scale=-1.0)
                        nc.vector.tensor_mul(
                            out=ctxT_sb[hh * 64:(hh + 1) * 64, gg, csl],
                            in0=un[0:HD, :], in1=rb)

            # ---- schedule ---------------------------------------------
            from itertools import chain

            def drain(filler):
                while pump(filler):
                    pass

            if 1 in phases and 2 in phases and 3 in phases:
                drain(iter(qkv_thunks(0)))
                regions = [
                    (0, 0, qkv_thunks(1)),
                    (0, 1, qkv_thunks(2)),
                    (0, 2, qkv_thunks(3)),
                    (0, 3, chain(qkv_thunks(4), qkv_thunks(5))),
                    (1, 0, qkv_thunks(6)),
                    (1, 1, qkv_thunks(7)),
                    (1, 2, chain(oproj_thunks(0), oproj_thunks(1),
                                 oproj_thunks(5))),
                    (1, 3, chain(oproj_thunks(2), oproj_thunks(3),
                                 oproj_thunks(4), oproj_thunks(6))),
                ]
                for b, c, filler in regions:
                    att_region(b, c, filler)
                drain(iter(oproj_thunks(7)))
            else:
                # debug path: sequential phases
                if 1 in phases:
                    for ch in range(NCH):
                        drain(iter(qkv_thunks(ch)))
                if 2 in phases:
                    for b in range(BSZ):
                        for c in range(NCK):
                            att_region(b, c, ())
                if 3 in phases:
                    for ch in range(NCH):
                        drain(iter(oproj_thunks(ch)))

    return nc


def _classify_mask(mask):
    m = np.asarray(mask, dtype=np.float32).reshape(SEQ, SEQ)
    if not np.any(m):
        return "none", None
    lower_ok = not np.any(m[np.tril_indices(SEQ)])
    upper = m[np.triu_indices(SEQ, 1)]
    if lower_ok and np.all(np.isneginf(upper)):
        return "causal", None
    return "general", np.ascontiguousarray(m.T)


def kernel(x, start_pos, freqs_cis, mask, wq_w, wq_b, wk_w, wk_b,
           wv_w, wv_b, wo_w, wo_b):
    global LAST_RESULTS
    _install_compile_hook()
    from concourse.bass_utils import run_bass_kernel_spmd

    x = np.asarray(x, dtype=np.float32)
    mask_mode, maskT = _classify_mask(mask)
    wq_b = np.asarray(wq_b, dtype=np.float32)
    wk_b = np.asarray(wk_b, dtype=np.float32)
    wv_b = np.asarray(wv_b, dtype=np.float32)
    wo_b = np.asarray(wo_b, dtype=np.float32)
    use_qkb = bool(np.any(wq_b) or np.any(wk_b))
    use_vb = bool(np.any(wv_b))

    nc = _build(mask_mode, use_qkb, use_vb)

    xT = np.ascontiguousarray(x.reshape(T, DIM).T).astype(BF16)
    wqT = np.asarray(wq_w, dtype=np.float32).T.astype(BF16)  # [D, D]
    wkT = np.asarray(wk_w, dtype=np.float32).T.astype(BF16)
    wvT = np.asarray(wv_w, dtype=np.float32).T.astype(BF16)
    wo = np.asarray(wo_w, dtype=np.float32)

    in_maps = []
    for c in range(NCORES):
        sl = slice(HSL * c, HSL * (c + 1))
        im = {
            "xt": xT,
            "wqt": np.ascontiguousarray(wqT[:, sl]),
            "wkt": np.ascontiguousarray(wkT[:, sl]),
            "wvt": np.ascontiguousarray(wvT[:, sl]),
            "wot": np.ascontiguousarray(wo[:, sl].T).astype(BF16),
        }
        if use_qkb:
            im["qb"] = np.ascontiguousarray(wq_b[sl])
            im["kb"] = np.ascontiguousarray(wk_b[sl])
        if use_vb:
            im["vb"] = np.ascontiguousarray(wv_b[sl])
        if mask_mode == "general":
            im["maskt"] = maskT
        in_maps.append(im)

    res = run_bass_kernel_spmd(nc, in_maps, core_ids=list(range(NCORES)))
    LAST_RESULTS = res

    acc = np.zeros((DIM, T), dtype=np.float32)
    for r in res.results:
        acc += np.asarray(r["outT"], dtype=np.float32)
    out = acc.T + wo_b[None, :]
    return out.reshape(BSZ, SEQ, DIM).astype(np.float32)


# revision 25
# speedup vs baseline: 1.1513x; 1.0174x over previous
"""Trainium2 Bass kernel for nn_Attention_4930622456197.

Multi-head causal attention (B=2, S=2048, D=2048, 32 heads x head_dim 64)
with QKVO projections, tensor-parallel over heads across 8 NeuronCores
(4 heads per core).

Per-core plan (all matmul inputs bf16, f32 PSUM accumulation):
  Phase 1  QKV projections from host-pretransposed x^T [D, T]:
           Q^T, K^T in [128 (=2 heads x 64 dims), group, T] layout;
           V in natural [tok, head, 65] layout with a ones column
           appended (row 64 of V_aug.T) so the P@V matmul also produces
           the softmax denominators for free.
  Phase 2  Flash-style causal attention in score-transposed layout
           S^T[s, q] (scores never touch HBM).  exp on ScalarE with the
           1/sqrt(hd) scale folded in; no max-subtraction (scores are
           O(+-8) here, exp is safe in fp32->bf16).  The diagonal
           128x128 block of each strip is masked post-exp with a
           precomputed upper-triangular 0/1 tile.  O^T accumulates in
           PSUM over k-tiles; the 64 ones-columns of V_aug broadcast the
           softmax denominator to PSUM partitions 64..127, and 1/den is
           computed as exp(-ln(den)) on ScalarE.
  Phase 3  Row-parallel output projection producing a partial
           out^T [D, T]; host sums the 8 partials, adds wo_b.

  Emission interleaves the three phases: QKV chunks and output-projection
  tiles are "filler" thunks pumped between attention j-steps, keeping the
  PE array dense (HAM stays at 2.4 GHz) while ScalarE runs the softmax
  exps of the two in-flight head-pair streams.

The harness calls kernel(**inputs) with the full (unsharded) inputs and
expects the full [2, 2048, 2048] float32 output.
"""

import numpy as np
import ml_dtypes

BSZ, SEQ, DIM, NH = 2, 2048, 2048, 32
HD = DIM // NH            # 64
NCORES = 8
HPC = NH // NCORES        # 4 heads per core
HSL = HPC * HD            # 256 head-dims per core
T = BSZ * SEQ             # 4096 flattened tokens
SCALE = 1.0 / float(np.sqrt(HD))
BF16 = ml_dtypes.bfloat16

NKT = DIM // 128          # 16 contraction tiles over model dim
NCH = T // 512            # 8 token chunks of 512
NJ = SEQ // 128           # 16 k-tiles per sequence
NCK = SEQ // 512          # 4 q-chunks per sequence

# Output partial dtype: float32 is safest for the cross-core sum;
# bfloat16 halves the output DMA traffic.
OUT_BF16 = True

LAST_RESULTS = None       # BassKernelResults of the most recent run (for test.py)


# This walrus build caps EVERY instruction (HW-decoded and sequencer alike)
# at one sync-wait, so the legalizer splits excess waits regardless of opcode.
_SEQ_OPCODES = set()
_wc_counter = [0]


def _legalize_bir_waits(bir_bytes):
    """This container's walrus accepts only ONE sync-wait on HW-decoded
    instruction structs ("Too many sync wait commands" otherwise), but Tile
    freely emits 2-3 waits per instruction.  Split excess waits into
    standalone same-engine EventSemaphore instructions placed immediately
    before the instruction — the sequencer executes them in order, so the
    dependency semantics are identical."""
    import json as _json

    d = _json.loads(bir_bytes)
    n_split = 0
    for f in d.get("functions", []):
        for blk in f.get("blocks", []):
            out = []
            for ins in blk.get("instructions", []):
                si = ins.get("sync_info")
                waits = (si or {}).get("on_wait") or []
                if si is not None and len(waits) > 1 and \
                        ins.get("opcode") not in _SEQ_OPCODES:
                    for w in waits[:-1]:
                        _wc_counter[0] += 1
                        out.append({
                            "debug": ins.get("debug", 0),
                            "engine": ins["engine"],
                            "ins": [], "outs": [],
                            "name": f"I-wc{_wc_counter[0]}",
                            "opcode": "EventSemaphore",
                            "sync_info": {"on_wait": [w], "on_update": []},
                        })
                        n_split += 1
                    si["on_wait"] = waits[-1:]
                out.append(ins)
            blk["instructions"] = out
    if n_split:
        print(f"[kernel] wait-legalizer: split {n_split} excess waits")
    return _json.dumps(d).encode()


_hook_installed = [False]


def _install_compile_hook():
    """Route every BIR->NEFF compile in this process through the wait
    legalizer (both the direct bass_utils path and the bass2jax/axon path)."""
    if _hook_installed[0]:
        return
    import concourse.bass_utils as bu

    orig = bu.compile_bir_kernel

    def patched(bir_json, tmpdir, neff_name="file.neff"):
        return orig(_legalize_bir_waits(bir_json), tmpdir, neff_name=neff_name)

    bu.compile_bir_kernel = patched
    try:
        import concourse.bass2jax as b2j
        b2j.compile_bir_kernel = patched
    except Exception:
        pass
    _hook_installed[0] = True


def _build(mask_mode, use_qkb, use_vb, phases=(1, 2, 3)):
    """Builds the Bass program. mask_mode: 'causal' | 'none' | 'general'.
    phases: debug knob to emit only a subset of the pipeline."""
    import concourse.bass as bass
    import concourse.mybir as mybir
    import concourse.tile as tile
    from concourse.masks import make_upper_triangular

    dt = mybir.dt
    f32 = dt.float32
    bf16 = dt.bfloat16
    Exp = mybir.ActivationFunctionType.Exp
    Ln = mybir.ActivationFunctionType.Ln
    Identity = mybir.ActivationFunctionType.Identity
    out_dt = bf16 if OUT_BF16 else f32

    causal = mask_mode == "causal"

    nc = bass.Bass()
    xT_d = nc.dram_tensor("xt", [DIM, T], bf16, kind="ExternalInput")
    wqT_d = nc.dram_tensor("wqt", [DIM, HSL], bf16, kind="ExternalInput")
    wkT_d = nc.dram_tensor("wkt", [DIM, HSL], bf16, kind="ExternalInput")
    wvT_d = nc.dram_tensor("wvt", [DIM, HSL], bf16, kind="ExternalInput")
    woT_d = nc.dram_tensor("wot", [HSL, DIM], bf16, kind="ExternalInput")
    outT_d = nc.dram_tensor("outT", [DIM, T], out_dt, kind="ExternalOutput")
    qb_d = kb_d = vb_d = maskT_d = None
    if use_qkb:
        qb_d = nc.dram_tensor("qb", [HSL], f32, kind="ExternalInput")
        kb_d = nc.dram_tensor("kb", [HSL], f32, kind="ExternalInput")
    if use_vb:
        vb_d = nc.dram_tensor("vb", [HSL], f32, kind="ExternalInput")
    if mask_mode == "general":
        maskT_d = nc.dram_tensor("maskt", [SEQ, SEQ], f32, kind="ExternalInput")

    # 3-D views with 128-partition-major layout
    xT_ap = xT_d[:].rearrange("(kt p) t -> p kt t", p=128)
    wq_ap = wqT_d[:].rearrange("(kt p) m -> p kt m", p=128)
    wk_ap = wkT_d[:].rearrange("(kt p) m -> p kt m", p=128)
    wv_ap = wvT_d[:].rearrange("(kt p) m -> p kt m", p=128)
    wo_ap = woT_d[:].rearrange("(g p) n -> p g n", p=128)
    outT_ap = outT_d[:].rearrange("(ot p) t -> p ot t", p=128)

    with tile.TileContext(nc) as tc:
        with (
            tc.tile_pool(name="singles", bufs=1) as singles,
            tc.tile_pool(name="xload", bufs=3) as xload,
            tc.tile_pool(name="work", bufs=4) as work,
            tc.tile_pool(name="outp", bufs=4) as outp,
            tc.tile_pool(name="psum", bufs=2, space="PSUM") as psum,
            tc.tile_pool(name="otps", bufs=4, space="PSUM") as otps,
        ):
            # ---- resident tensors -------------------------------------
            wq_sb = singles.tile([128, NKT, HSL], bf16)
            wk_sb = singles.tile([128, NKT, HSL], bf16)
            wv_sb = singles.tile([128, NKT, HSL], bf16)
            wo_sb = singles.tile([128, 2, DIM], bf16)
            # wq is issued first so the very first Q-projection matmul can
            # start as early as possible; wk/wv/wo are issued from inside
            # qkv_thunks(0) right after the first x-chunk quarters.
            for q in range(4):
                ksl = slice(4 * q, 4 * q + 4)
                nc.sync.dma_start(out=wq_sb[:, ksl], in_=wq_ap[:, ksl])

            qt_sb = singles.tile([128, 2, T], bf16)
            kt_sb = singles.tile([128, 2, T], bf16)
            ctxT_sb = singles.tile([128, 2, T], bf16)
            # V with 64 ones-columns per head: the P@V matmul then writes the
            # softmax denominator to PSUM partitions 64..127 (a free
            # cross-partition broadcast).
            vaug_sb = singles.tile([128, T // 128, HPC, 2 * HD], bf16)
            nc.vector.memset(vaug_sb, 1.0)

            qb_sb = kb_sb = vb_bc = None
            if use_qkb:
                qb_sb = singles.tile([128, 2], f32)
                kb_sb = singles.tile([128, 2], f32)
                nc.sync.dma_start(out=qb_sb, in_=qb_d[:].rearrange("(g p) -> p g", p=128))
                nc.sync.dma_start(out=kb_sb, in_=kb_d[:].rearrange("(g p) -> p g", p=128))
            if use_vb:
                vb_bc = singles.tile([128, HSL], f32)
                nc.sync.dma_start(out=vb_bc, in_=vb_d[:].to_broadcast([128, HSL]))

            triu_sb = None
            if causal:
                triu_sb = singles.tile([128, 128], bf16)
                make_upper_triangular(nc, triu_sb, val=1.0, diag=True)

            # ---- emission units ---------------------------------------
            # QKV projections and the output projection are emitted as
            # "filler" thunks interleaved between attention j-steps, so PE
            # always has independent matmul work while ScalarE runs the
            # softmax exps of the in-flight attention streams.

            def qkv_thunks(ch):
                tsl = slice(ch * 512, (ch + 1) * 512)
                xt_box = []

                def load():
                    xt_ch = xload.tile([128, NKT, 512], bf16, tag="xt")
                    for q in range(4):
                        ksl = slice(4 * q, 4 * q + 4)
                        nc.sync.dma_start(out=xt_ch[:, ksl],
                                          in_=xT_ap[:, ksl, tsl])
                    xt_box.append(xt_ch)
                    if ch == 0:
                        for q in range(4):
                            ksl = slice(4 * q, 4 * q + 4)
                            nc.sync.dma_start(out=wk_sb[:, ksl],
                                              in_=wk_ap[:, ksl])
                            nc.sync.dma_start(out=wv_sb[:, ksl],
                                              in_=wv_ap[:, ksl])
                        nc.sync.dma_start(out=wo_sb, in_=wo_ap)
                yield load

                # both head-pair groups of one projection share a single
                # 2-bank PSUM tile (halves filler slot pressure in the st2
                # rotation) and evict with one instruction
                def qk_pair(w_sb, dst_sb, b_sb):
                    ps2 = psum.tile([128, 1024], f32, tag="st2", name="qk2")
                    for g in range(2):
                        for k in range(NKT):
                            nc.tensor.matmul(
                                ps2[:, g * 512:(g + 1) * 512],
                                lhsT=w_sb[:, k, g * 128:(g + 1) * 128],
                                rhs=xt_box[0][:, k, :],
                                start=(k == 0), stop=(k == NKT - 1))
                    if b_sb is not None:
                        for g in range(2):
                            nc.scalar.activation(
                                out=dst_sb[:, g, tsl],
                                in_=ps2[:, g * 512:(g + 1) * 512],
                                func=Identity, bias=b_sb[:, g:g + 1], scale=1.0)
                    else:
                        nc.vector.tensor_copy(
                            out=dst_sb[:, :, tsl],
                            in_=ps2.rearrange("p (g n) -> p g n", g=2))

                def v_pair(tp):
                    ps2 = psum.tile([128, 1024], f32, tag="st2", name="v2")
                    for i in range(2):
                        tt = 2 * tp + i
                        for k in range(NKT):
                            nc.tensor.matmul(
                                ps2[:, i * 512:i * 512 + HSL],
                                lhsT=xt_box[0][:, k, tt * 128:(tt + 1) * 128],
                                rhs=wv_sb[:, k, :],
                                start=(k == 0), stop=(k == NKT - 1))
                    tg0 = ch * 4 + 2 * tp
                    vdst = vaug_sb[:, tg0:tg0 + 2, :, 0:HD]
                    vsrc = ps2.rearrange("p (i n) -> p i n", i=2)[:, :, 0:HSL]
                    vsrc = vsrc.rearrange("p i (h m) -> p i h m", h=HPC)
                    if vb_bc is not None:
                        nc.vector.tensor_add(
                            out=vdst, in0=vsrc,
                            in1=vb_bc[:, None, :].to_broadcast(
                                [128, 2, HSL]).rearrange(
                                "p i (h m) -> p i h m", h=HPC))
                    else:
                        nc.vector.tensor_copy(out=vdst, in_=vsrc)

                import functools
                yield functools.partial(qk_pair, wq_sb, qt_sb, qb_sb)
                yield functools.partial(qk_pair, wk_sb, kt_sb, kb_sb)
                for tp in range(2):
                    yield functools.partial(v_pair, tp)

            def oproj_thunks(ch):
                import functools
                tsl = slice(ch * 512, (ch + 1) * 512)

                def o_pair(op):
                    ps2 = psum.tile([128, 1024], f32, tag="st2", name="o2")
                    for i in range(2):
                        o = 2 * op + i
                        for g2 in range(2):
                            nc.tensor.matmul(
                                ps2[:, i * 512:(i + 1) * 512],
                                lhsT=wo_sb[:, g2, o * 128:(o + 1) * 128],
                                rhs=ctxT_sb[:, g2, tsl],
                                start=(g2 == 0), stop=(g2 == 1))
                    osb = outp.tile([128, 2, 512], out_dt, tag="out_sb")
                    src2 = ps2.rearrange("p (i n) -> p i n", i=2)
                    if op % 2 == 0:
                        nc.vector.tensor_copy(out=osb, in_=src2)
                    else:
                        nc.scalar.copy(out=osb, in_=src2)
                    nc.sync.dma_start(
                        out=outT_ap[:, 2 * op:2 * op + 2, tsl], in_=osb)

                for op in range(DIM // 256):
                    yield functools.partial(o_pair, op)

            def pump(filler, n=1):
                for _ in range(n):
                    t = next(filler, None)
                    if t is None:
                        return False
                    t()
                return True

            def att_region(b, c, filler):
                """Attention for one (batch, q-chunk): head-pair streams g=0,1
                interleaved per j-step; O^T matmuls lag 2 steps; filler thunks
                are spread over the j-steps with a few reserved to bridge the
                region boundary while ScalarE drains the last exps."""
                thunks = list(filler)
                reserve = thunks[-5:]
                body = thunks[:-5]
                bi = [0]
                ots = {}
                for gg in range(2):
                    ots[gg, 0] = otps.tile([128, 512], f32, tag="ot", name="otA")
                    ots[gg, 1] = otps.tile([128, 512], f32, tag="ot", name="otB")
                jmax = 4 * c + 4 if causal else NJ
                pend = []

                def flush_ot(gg, j, qo, pt2):
                    for hh in range(2):
                        nc.tensor.matmul(
                            ots[gg, hh][:, qo:512],
                            lhsT=vaug_sb[:, b * NJ + j, 2 * gg + hh, :],
                            rhs=pt2[:, 512 * hh + qo:512 * hh + 512],
                            start=(j == 0), stop=(j == jmax - 1))

                for j in range(jmax):
                    qo = max(0, j * 128 - c * 512) if causal else 0
                    ssl = slice(b * SEQ + j * 128, b * SEQ + (j + 1) * 128)
                    qsl = slice(b * SEQ + c * 512 + qo, b * SEQ + (c + 1) * 512)
                    for gg in range(2):
                        st2 = psum.tile([128, 1024], f32, tag="st2", name="st2")
                        nc.tensor.matmul(
                            st2[:, qo:512], lhsT=kt_sb[0:64, gg, ssl],
                            rhs=qt_sb[0:64, gg, qsl],
                            start=True, stop=True, tile_position=(0, 0))
                        nc.tensor.matmul(
                            st2[:, 512 + qo:1024], lhsT=kt_sb[64:128, gg, ssl],
                            rhs=qt_sb[64:128, gg, qsl],
                            start=True, stop=True, tile_position=(64, 0))
                        if maskT_d is not None:
                            mt = work.tile([128, 512], f32, tag="mt")
                            nc.sync.dma_start(
                                out=mt,
                                in_=maskT_d[j * 128:(j + 1) * 128,
                                            c * 512:(c + 1) * 512])
                            for hh in range(2):
                                sl = slice(512 * hh, 512 * hh + 512)
                                nc.vector.tensor_add(
                                    out=st2[:, sl], in0=st2[:, sl], in1=mt)
                        pt2 = work.tile([128, 1024], bf16, tag="pt", bufs=8)
                        nc.scalar.activation(
                            out=pt2.rearrange("p (two n) -> p two n", two=2)[:, :, qo:512],
                            in_=st2.rearrange("p (two n) -> p two n", two=2)[:, :, qo:512],
                            func=Exp, scale=SCALE)
                        if causal and j * 128 >= c * 512:
                            dv = pt2.rearrange("p (two n) -> p two n", two=2)[:, :, qo:qo + 128]
                            nc.vector.tensor_mul(
                                out=dv, in0=dv,
                                in1=triu_sb[:, None, :].to_broadcast([128, 2, 128]))
                        pend.append((gg, j, qo, pt2))
                        while len(pend) > 6:
                            flush_ot(*pend.pop(0))
                    want = ((j + 1) * len(body) + jmax - 1) // jmax
                    while bi[0] < min(want, len(body)):
                        body[bi[0]]()
                        bi[0] += 1
                while pend:
                    flush_ot(*pend.pop(0))
                for t in reserve:
                    t()
                # chunk end: one f32 copy frees each accumulator slot; the
                # Ln/Exp reciprocal + multiply then run from SBUF overlapped
                # with the next region.
                for gg in range(2):
                    csl = slice(b * SEQ + c * 512, b * SEQ + (c + 1) * 512)
                    for hh in range(2):
                        ot = ots[gg, hh]
                        un = work.tile([128, 512], f32, tag="unctx")
                        nc.vector.tensor_copy(out=un, in_=ot)
                        rb = work.tile([64, 512], f32, tag="rb")
                        nc.scalar.activation(out=rb, in_=un[HD:2 * HD, :],
                                             func=Ln, scale=1.0)
                        nc.scalar.activation(out=rb, in_=rb,
                                             func=Exp, scale=-1.0)
                        nc.vector.tensor_mul(
                            out=ctxT_sb[hh * 64:(hh + 1) * 64, gg, csl],
                            in0=un[0:HD, :], in1=rb)

            # ---- schedule ---------------------------------------------
            from itertools import chain

            def drain(filler):
                while pump(filler):
                    pass

            if 1 in phases and 2 in phases and 3 in phases:
                drain(iter(qkv_thunks(0)))
                regions = [
                    (0, 0, qkv_thunks(1)),
                    (0, 1, qkv_thunks(2)),
                    (0, 2, qkv_thunks(3)),
                    (0, 3, chain(qkv_thunks(4), qkv_thunks(5))),
                    (1, 0, qkv_thunks(6)),
                    (1, 1, qkv_thunks(7)),
                    (1, 2, chain(oproj_thunks(0), oproj_thunks(1),
                                 oproj_thunks(5))),
                    (1, 3, chain(oproj_thunks(2), oproj_thunks(3),
                                 oproj_thunks(4), oproj_thunks(6))),
                ]
                for b, c, filler in regions:
                    att_region(b, c, filler)
                drain(iter(oproj_thunks(7)))
            else:
                # debug path: sequential phases
                if 1 in phases:
                    for ch in range(NCH):
                        drain(iter(qkv_thunks(ch)))
                if 2 in phases:
                    for b in range(BSZ):
                        for c in range(NCK):
                            att_region(b, c, ())
                if 3 in phases:
                    for ch in range(NCH):
                        drain(iter(oproj_thunks(ch)))

    return nc


def _classify_mask(mask):
    m = np.asarray(mask, dtype=np.float32).reshape(SEQ, SEQ)
    if not np.any(m):
        return "none", None
    lower_ok = not np.any(m[np.tril_indices(SEQ)])
    upper = m[np.triu_indices(SEQ, 1)]
    if lower_ok and np.all(np.isneginf(upper)):
        return "causal", None
    return "general", np.ascontiguousarray(m.T)


def kernel(x, start_pos, freqs_cis, mask, wq_w, wq_b, wk_w, wk_b,
           wv_w, wv_b, wo_w, wo_b):
    global LAST_RESULTS
    _install_compile_hook()
    from concourse.bass_utils import run_bass_kernel_spmd

    x = np.asarray(x, dtype=np.float32)
    mask_mode, maskT = _classify_mask(mask)
    wq_b = np.asarray(wq_b, dtype=np.float32)
    wk_b = np.asarray(wk_b, dtype=np.float32)
    wv_b = np.asarray(wv_b, dtype=np.float32)
    wo_b = np.asarray(wo_b, dtype=np.float32)
    use_qkb = bool(np.any(wq_b) or np.any(wk_b))
    use_vb = bool(np.any(wv_b))

    nc = _build(mask_mode, use_qkb, use_vb)

    xT = np.ascontiguousarray(x.reshape(T, DIM).T).astype(BF16)
    wqT = np.asarray(wq_w, dtype=np.float32).T.astype(BF16)  # [D, D]
    wkT = np.asarray(wk_w, dtype=np.float32).T.astype(BF16)
    wvT = np.asarray(wv_w, dtype=np.float32).T.astype(BF16)
    wo = np.asarray(wo_w, dtype=np.float32)

    in_maps = []
    for c in range(NCORES):
        sl = slice(HSL * c, HSL * (c + 1))
        im = {
            "xt": xT,
            "wqt": np.ascontiguousarray(wqT[:, sl]),
            "wkt": np.ascontiguousarray(wkT[:, sl]),
            "wvt": np.ascontiguousarray(wvT[:, sl]),
            "wot": np.ascontiguousarray(wo[:, sl].T).astype(BF16),
        }
        if use_qkb:
            im["qb"] = np.ascontiguousarray(wq_b[sl])
            im["kb"] = np.ascontiguousarray(wk_b[sl])
        if use_vb:
            im["vb"] = np.ascontiguousarray(wv_b[sl])
        if mask_mode == "general":
            im["maskt"] = maskT
        in_maps.append(im)

    res = run_bass_kernel_spmd(nc, in_maps, core_ids=list(range(NCORES)))
    LAST_RESULTS = res

    acc = np.zeros((DIM, T), dtype=np.float32)
    for r in res.results:
        acc += np.asarray(r["outT"], dtype=np.float32)
    out = acc.T + wo_b[None, :]
    return out.reshape(BSZ, SEQ, DIM).astype(np.float32)


# revision 26
# speedup vs baseline: 1.1598x; 1.0074x over previous
"""Trainium2 Bass kernel for nn_Attention_4930622456197.

Multi-head causal attention (B=2, S=2048, D=2048, 32 heads x head_dim 64)
with QKVO projections, tensor-parallel over heads across 8 NeuronCores
(4 heads per core).

Per-core plan (all matmul inputs bf16, f32 PSUM accumulation):
  Phase 1  QKV projections from host-pretransposed x^T [D, T]:
           Q^T, K^T in [128 (=2 heads x 64 dims), group, T] layout;
           V in natural [tok, head, 65] layout with a ones column
           appended (row 64 of V_aug.T) so the P@V matmul also produces
           the softmax denominators for free.
  Phase 2  Flash-style causal attention in score-transposed layout
           S^T[s, q] (scores never touch HBM).  exp on ScalarE with the
           1/sqrt(hd) scale folded in; no max-subtraction (scores are
           O(+-8) here, exp is safe in fp32->bf16).  The diagonal
           128x128 block of each strip is masked post-exp with a
           precomputed upper-triangular 0/1 tile.  O^T accumulates in
           PSUM over k-tiles; the 64 ones-columns of V_aug broadcast the
           softmax denominator to PSUM partitions 64..127, and 1/den is
           computed as exp(-ln(den)) on ScalarE.
  Phase 3  Row-parallel output projection producing a partial
           out^T [D, T]; host sums the 8 partials, adds wo_b.

  Emission interleaves the three phases: QKV chunks and output-projection
  tiles are "filler" thunks pumped between attention j-steps, keeping the
  PE array dense (HAM stays at 2.4 GHz) while ScalarE runs the softmax
  exps of the two in-flight head-pair streams.

The harness calls kernel(**inputs) with the full (unsharded) inputs and
expects the full [2, 2048, 2048] float32 output.
"""

import numpy as np
import ml_dtypes

BSZ, SEQ, DIM, NH = 2, 2048, 2048, 32
HD = DIM // NH            # 64
NCORES = 8
HPC = NH // NCORES        # 4 heads per core
HSL = HPC * HD            # 256 head-dims per core
T = BSZ * SEQ             # 4096 flattened tokens
SCALE = 1.0 / float(np.sqrt(HD))
BF16 = ml_dtypes.bfloat16

NKT = DIM // 128          # 16 contraction tiles over model dim
NCH = T // 512            # 8 token chunks of 512
NJ = SEQ // 128           # 16 k-tiles per sequence
NCK = SEQ // 512          # 4 q-chunks per sequence

# Output partial dtype: float32 is safest for the cross-core sum;
# bfloat16 halves the output DMA traffic.
OUT_BF16 = True

LAST_RESULTS = None       # BassKernelResults of the most recent run (for test.py)


# This walrus build caps EVERY instruction (HW-decoded and sequencer alike)
# at one sync-wait, so the legalizer splits excess waits regardless of opcode.
_SEQ_OPCODES = set()
_wc_counter = [0]


def _legalize_bir_waits(bir_bytes):
    """This container's walrus accepts only ONE sync-wait on HW-decoded
    instruction structs ("Too many sync wait commands" otherwise), but Tile
    freely emits 2-3 waits per instruction.  Split excess waits into
    standalone same-engine EventSemaphore instructions placed immediately
    before the instruction — the sequencer executes them in order, so the
    dependency semantics are identical."""
    import json as _json

    d = _json.loads(bir_bytes)
    n_split = 0
    for f in d.get("functions", []):
        for blk in f.get("blocks", []):
            out = []
            for ins in blk.get("instructions", []):
                si = ins.get("sync_info")
                waits = (si or {}).get("on_wait") or []
                if si is not None and len(waits) > 1 and \
                        ins.get("opcode") not in _SEQ_OPCODES:
                    for w in waits[:-1]:
                        _wc_counter[0] += 1
                        out.append({
                            "debug": ins.get("debug", 0),
                            "engine": ins["engine"],
                            "ins": [], "outs": [],
                            "name": f"I-wc{_wc_counter[0]}",
                            "opcode": "EventSemaphore",
                            "sync_info": {"on_wait": [w], "on_update": []},
                        })
                        n_split += 1
                    si["on_wait"] = waits[-1:]
                out.append(ins)
            blk["instructions"] = out
    if n_split:
        print(f"[kernel] wait-legalizer: split {n_split} excess waits")
    return _json.dumps(d).encode()


_hook_installed = [False]


def _install_compile_hook():
    """Route every BIR->NEFF compile in this process through the wait
    legalizer (both the direct bass_utils path and the bass2jax/axon path)."""
    if _hook_installed[0]:
        return
    import concourse.bass_utils as bu

    orig = bu.compile_bir_kernel

    def patched(bir_json, tmpdir, neff_name="file.neff"):
        return orig(_legalize_bir_waits(bir_json), tmpdir, neff_name=neff_name)

    bu.compile_bir_kernel = patched
    try:
        import concourse.bass2jax as b2j
        b2j.compile_bir_kernel = patched
    except Exception:
        pass
    _hook_installed[0] = True


def _build(mask_mode, use_qkb, use_vb, phases=(1, 2, 3)):
    """Builds the Bass program. mask_mode: 'causal' | 'none' | 'general'.
    phases: debug knob to emit only a subset of the pipeline."""
    import concourse.bass as bass
    import concourse.mybir as mybir
    import concourse.tile as tile
    from concourse.masks import make_upper_triangular

    dt = mybir.dt
    f32 = dt.float32
    bf16 = dt.bfloat16
    Exp = mybir.ActivationFunctionType.Exp
    Ln = mybir.ActivationFunctionType.Ln
    Identity = mybir.ActivationFunctionType.Identity
    out_dt = bf16 if OUT_BF16 else f32

    causal = mask_mode == "causal"

    nc = bass.Bass()
    xT_d = nc.dram_tensor("xt", [DIM, T], bf16, kind="ExternalInput")
    wqT_d = nc.dram_tensor("wqt", [DIM, HSL], bf16, kind="ExternalInput")
    wkT_d = nc.dram_tensor("wkt", [DIM, HSL], bf16, kind="ExternalInput")
    wvT_d = nc.dram_tensor("wvt", [DIM, HSL], bf16, kind="ExternalInput")
    woT_d = nc.dram_tensor("wot", [HSL, DIM], bf16, kind="ExternalInput")
    outT_d = nc.dram_tensor("outT", [DIM, T], out_dt, kind="ExternalOutput")
    qb_d = kb_d = vb_d = maskT_d = None
    if use_qkb:
        qb_d = nc.dram_tensor("qb", [HSL], f32, kind="ExternalInput")
        kb_d = nc.dram_tensor("kb", [HSL], f32, kind="ExternalInput")
    if use_vb:
        vb_d = nc.dram_tensor("vb", [HSL], f32, kind="ExternalInput")
    if mask_mode == "general":
        maskT_d = nc.dram_tensor("maskt", [SEQ, SEQ], f32, kind="ExternalInput")

    # 3-D views with 128-partition-major layout
    xT_ap = xT_d[:].rearrange("(kt p) t -> p kt t", p=128)
    wq_ap = wqT_d[:].rearrange("(kt p) m -> p kt m", p=128)
    wk_ap = wkT_d[:].rearrange("(kt p) m -> p kt m", p=128)
    wv_ap = wvT_d[:].rearrange("(kt p) m -> p kt m", p=128)
    wo_ap = woT_d[:].rearrange("(g p) n -> p g n", p=128)
    outT_ap = outT_d[:].rearrange("(ot p) t -> p ot t", p=128)

    with tile.TileContext(nc) as tc:
        with (
            tc.tile_pool(name="singles", bufs=1) as singles,
            tc.tile_pool(name="xload", bufs=3) as xload,
            tc.tile_pool(name="work", bufs=4) as work,
            tc.tile_pool(name="outp", bufs=4) as outp,
            tc.tile_pool(name="psum", bufs=2, space="PSUM") as psum,
            tc.tile_pool(name="otps", bufs=4, space="PSUM") as otps,
        ):
            # ---- resident tensors -------------------------------------
            wq_sb = singles.tile([128, NKT, HSL], bf16)
            wk_sb = singles.tile([128, NKT, HSL], bf16)
            wv_sb = singles.tile([128, NKT, HSL], bf16)
            wo_sb = singles.tile([128, 2, DIM], bf16)
            # wq is issued first so the very first Q-projection matmul can
            # start as early as possible; wk/wv/wo are issued from inside
            # qkv_thunks(0) right after the first x-chunk quarters.
            for q in range(4):
                ksl = slice(4 * q, 4 * q + 4)
                nc.sync.dma_start(out=wq_sb[:, ksl], in_=wq_ap[:, ksl])

            qt_sb = singles.tile([128, 2, T], bf16)
            kt_sb = singles.tile([128, 2, T], bf16)
            ctxT_sb = singles.tile([128, 2, T], bf16)
            # V with 64 ones-columns per head: the P@V matmul then writes the
            # softmax denominator to PSUM partitions 64..127 (a free
            # cross-partition broadcast).
            vaug_sb = singles.tile([128, T // 128, HPC, 2 * HD], bf16)
            nc.vector.memset(vaug_sb, 1.0)

            qb_sb = kb_sb = vb_bc = None
            if use_qkb:
                qb_sb = singles.tile([128, 2], f32)
                kb_sb = singles.tile([128, 2], f32)
                nc.sync.dma_start(out=qb_sb, in_=qb_d[:].rearrange("(g p) -> p g", p=128))
                nc.sync.dma_start(out=kb_sb, in_=kb_d[:].rearrange("(g p) -> p g", p=128))
            if use_vb:
                vb_bc = singles.tile([128, HSL], f32)
                nc.sync.dma_start(out=vb_bc, in_=vb_d[:].to_broadcast([128, HSL]))

            triu_sb = None
            if causal:
                triu_sb = singles.tile([128, 128], bf16)
                make_upper_triangular(nc, triu_sb, val=1.0, diag=True)

            # ---- emission units ---------------------------------------
            # QKV projections and the output projection are emitted as
            # "filler" thunks interleaved between attention j-steps, so PE
            # always has independent matmul work while ScalarE runs the
            # softmax exps of the in-flight attention streams.

            def qkv_thunks(ch):
                tsl = slice(ch * 512, (ch + 1) * 512)
                xt_box = []

                def load():
                    xt_ch = xload.tile([128, NKT, 512], bf16, tag="xt")
                    for q in range(4):
                        ksl = slice(4 * q, 4 * q + 4)
                        nc.sync.dma_start(out=xt_ch[:, ksl],
                                          in_=xT_ap[:, ksl, tsl])
                    xt_box.append(xt_ch)
                    if ch == 0:
                        for q in range(4):
                            ksl = slice(4 * q, 4 * q + 4)
                            nc.sync.dma_start(out=wk_sb[:, ksl],
                                              in_=wk_ap[:, ksl])
                            nc.sync.dma_start(out=wv_sb[:, ksl],
                                              in_=wv_ap[:, ksl])
                        nc.sync.dma_start(out=wo_sb, in_=wo_ap)
                yield load

                # both head-pair groups of one projection share a single
                # 2-bank PSUM tile (halves filler slot pressure in the st2
                # rotation) and evict with one instruction
                def qk_pair(w_sb, dst_sb, b_sb):
                    ps2 = psum.tile([128, 1024], f32, tag="st2", name="qk2")
                    for g in range(2):
                        for k in range(NKT):
                            nc.tensor.matmul(
                                ps2[:, g * 512:(g + 1) * 512],
                                lhsT=w_sb[:, k, g * 128:(g + 1) * 128],
                                rhs=xt_box[0][:, k, :],
                                start=(k == 0), stop=(k == NKT - 1))
                    if b_sb is not None:
                        for g in range(2):
                            nc.scalar.activation(
                                out=dst_sb[:, g, tsl],
                                in_=ps2[:, g * 512:(g + 1) * 512],
                                func=Identity, bias=b_sb[:, g:g + 1], scale=1.0)
                    else:
                        nc.vector.tensor_copy(
                            out=dst_sb[:, :, tsl],
                            in_=ps2.rearrange("p (g n) -> p g n", g=2))

                def v_pair(tp):
                    ps2 = psum.tile([128, 1024], f32, tag="st2", name="v2")
                    for i in range(2):
                        tt = 2 * tp + i
                        for k in range(NKT):
                            nc.tensor.matmul(
                                ps2[:, i * 512:i * 512 + HSL],
                                lhsT=xt_box[0][:, k, tt * 128:(tt + 1) * 128],
                                rhs=wv_sb[:, k, :],
                                start=(k == 0), stop=(k == NKT - 1))
                    tg0 = ch * 4 + 2 * tp
                    vdst = vaug_sb[:, tg0:tg0 + 2, :, 0:HD]
                    vsrc = ps2.rearrange("p (i n) -> p i n", i=2)[:, :, 0:HSL]
                    vsrc = vsrc.rearrange("p i (h m) -> p i h m", h=HPC)
                    if vb_bc is not None:
                        nc.vector.tensor_add(
                            out=vdst, in0=vsrc,
                            in1=vb_bc[:, None, :].to_broadcast(
                                [128, 2, HSL]).rearrange(
                                "p i (h m) -> p i h m", h=HPC))
                    else:
                        nc.vector.tensor_copy(out=vdst, in_=vsrc)

                import functools
                yield functools.partial(qk_pair, wq_sb, qt_sb, qb_sb)
                yield functools.partial(qk_pair, wk_sb, kt_sb, kb_sb)
                for tp in range(2):
                    yield functools.partial(v_pair, tp)

            def oproj_thunks(ch):
                import functools
                tsl = slice(ch * 512, (ch + 1) * 512)

                def o_pair(op):
                    ps2 = psum.tile([128, 1024], f32, tag="st2", name="o2")
                    for i in range(2):
                        o = 2 * op + i
                        for g2 in range(2):
                            nc.tensor.matmul(
                                ps2[:, i * 512:(i + 1) * 512],
                                lhsT=wo_sb[:, g2, o * 128:(o + 1) * 128],
                                rhs=ctxT_sb[:, g2, tsl],
                                start=(g2 == 0), stop=(g2 == 1))
                    osb = outp.tile([128, 2, 512], out_dt, tag="out_sb")
                    src2 = ps2.rearrange("p (i n) -> p i n", i=2)
                    if op % 2 == 0:
                        nc.vector.tensor_copy(out=osb, in_=src2)
                    else:
                        nc.scalar.copy(out=osb, in_=src2)
                    nc.sync.dma_start(
                        out=outT_ap[:, 2 * op:2 * op + 2, tsl], in_=osb)

                for op in range(DIM // 256):
                    yield functools.partial(o_pair, op)

            def pump(filler, n=1):
                for _ in range(n):
                    t = next(filler, None)
                    if t is None:
                        return False
                    t()
                return True

            def att_region(b, c, filler):
                """Attention for one (batch, q-chunk): head-pair streams g=0,1
                interleaved per j-step; O^T matmuls lag 2 steps; filler thunks
                are spread over the j-steps with a few reserved to bridge the
                region boundary while ScalarE drains the last exps."""
                thunks = list(filler)
                reserve = thunks[-7:]
                body = thunks[:-7]
                bi = [0]
                ots = {}
                for gg in range(2):
                    ots[gg, 0] = otps.tile([128, 512], f32, tag="ot", name="otA")
                    ots[gg, 1] = otps.tile([128, 512], f32, tag="ot", name="otB")
                jmax = 4 * c + 4 if causal else NJ
                pend = []

                def flush_ot(gg, j, qo, pt2):
                    for hh in range(2):
                        nc.tensor.matmul(
                            ots[gg, hh][:, qo:512],
                            lhsT=vaug_sb[:, b * NJ + j, 2 * gg + hh, :],
                            rhs=pt2[:, 512 * hh + qo:512 * hh + 512],
                            start=(j == 0), stop=(j == jmax - 1))

                for j in range(jmax):
                    qo = max(0, j * 128 - c * 512) if causal else 0
                    ssl = slice(b * SEQ + j * 128, b * SEQ + (j + 1) * 128)
                    qsl = slice(b * SEQ + c * 512 + qo, b * SEQ + (c + 1) * 512)
                    for gg in range(2):
                        st2 = psum.tile([128, 1024], f32, tag="st2", name="st2")
                        nc.tensor.matmul(
                            st2[:, qo:512], lhsT=kt_sb[0:64, gg, ssl],
                            rhs=qt_sb[0:64, gg, qsl],
                            start=True, stop=True, tile_position=(0, 0))
                        nc.tensor.matmul(
                            st2[:, 512 + qo:1024], lhsT=kt_sb[64:128, gg, ssl],
                            rhs=qt_sb[64:128, gg, qsl],
                            start=True, stop=True, tile_position=(64, 0))
                        if maskT_d is not None:
                            mt = work.tile([128, 512], f32, tag="mt")
                            nc.sync.dma_start(
                                out=mt,
                                in_=maskT_d[j * 128:(j + 1) * 128,
                                            c * 512:(c + 1) * 512])
                            for hh in range(2):
                                sl = slice(512 * hh, 512 * hh + 512)
                                nc.vector.tensor_add(
                                    out=st2[:, sl], in0=st2[:, sl], in1=mt)
                        pt2 = work.tile([128, 1024], bf16, tag="pt", bufs=8)
                        nc.scalar.activation(
                            out=pt2.rearrange("p (two n) -> p two n", two=2)[:, :, qo:512],
                            in_=st2.rearrange("p (two n) -> p two n", two=2)[:, :, qo:512],
                            func=Exp, scale=SCALE)
                        if causal and j * 128 >= c * 512:
                            dv = pt2.rearrange("p (two n) -> p two n", two=2)[:, :, qo:qo + 128]
                            nc.vector.tensor_mul(
                                out=dv, in0=dv,
                                in1=triu_sb[:, None, :].to_broadcast([128, 2, 128]))
                        pend.append((gg, j, qo, pt2))
                        while len(pend) > 6:
                            flush_ot(*pend.pop(0))
                    want = ((j + 1) * len(body) + jmax - 1) // jmax
                    while bi[0] < min(want, len(body)):
                        body[bi[0]]()
                        bi[0] += 1
                while pend:
                    flush_ot(*pend.pop(0))
                for t in reserve:
                    t()
                # chunk end: one f32 copy frees each accumulator slot; the
                # Ln/Exp reciprocal + multiply then run from SBUF overlapped
                # with the next region.
                for gg in range(2):
                    csl = slice(b * SEQ + c * 512, b * SEQ + (c + 1) * 512)
                    for hh in range(2):
                        ot = ots[gg, hh]
                        un = work.tile([128, 512], f32, tag="unctx")
                        nc.vector.tensor_copy(out=un, in_=ot)
                        rb = work.tile([64, 512], f32, tag="rb")
                        nc.scalar.activation(out=rb, in_=un[HD:2 * HD, :],
                                             func=Ln, scale=1.0)
                        nc.scalar.activation(out=rb, in_=rb,
                                             func=Exp, scale=-1.0)
                        nc.vector.tensor_mul(
                            out=ctxT_sb[hh * 64:(hh + 1) * 64, gg, csl],
                            in0=un[0:HD, :], in1=rb)

            # ---- schedule ---------------------------------------------
            from itertools import chain

            def drain(filler):
                while pump(filler):
                    pass

            if 1 in phases and 2 in phases and 3 in phases:
                drain(iter(qkv_thunks(0)))
                regions = [
                    (0, 0, qkv_thunks(1)),
                    (0, 1, qkv_thunks(2)),
                    (0, 2, qkv_thunks(3)),
                    (0, 3, chain(qkv_thunks(4), qkv_thunks(5))),
                    (1, 0, qkv_thunks(6)),
                    (1, 1, qkv_thunks(7)),
                    (1, 2, chain(oproj_thunks(0), oproj_thunks(1),
                                 oproj_thunks(5))),
                    (1, 3, chain(oproj_thunks(2), oproj_thunks(3),
                                 oproj_thunks(4), oproj_thunks(6))),
                ]
                for b, c, filler in regions:
                    att_region(b, c, filler)
                drain(iter(oproj_thunks(7)))
            else:
                # debug path: sequential phases
                if 1 in phases:
                    for ch in range(NCH):
                        drain(iter(qkv_thunks(ch)))
                if 2 in phases:
                    for b in range(BSZ):
                        for c in range(NCK):
                            att_region(b, c, ())
                if 3 in phases:
                    for ch in range(NCH):
                        drain(iter(oproj_thunks(ch)))

    return nc


def _classify_mask(mask):
    m = np.asarray(mask, dtype=np.float32).reshape(SEQ, SEQ)
    if not np.any(m):
        return "none", None
    lower_ok = not np.any(m[np.tril_indices(SEQ)])
    upper = m[np.triu_indices(SEQ, 1)]
    if lower_ok and np.all(np.isneginf(upper)):
        return "causal", None
    return "general", np.ascontiguousarray(m.T)


def kernel(x, start_pos, freqs_cis, mask, wq_w, wq_b, wk_w, wk_b,
           wv_w, wv_b, wo_w, wo_b):
    global LAST_RESULTS
    _install_compile_hook()
    from concourse.bass_utils import run_bass_kernel_spmd

    x = np.asarray(x, dtype=np.float32)
    mask_mode, maskT = _classify_mask(mask)
    wq_b = np.asarray(wq_b, dtype=np.float32)
    wk_b = np.asarray(wk_b, dtype=np.float32)
    wv_b = np.asarray(wv_b, dtype=np.float32)
    wo_b = np.asarray(wo_b, dtype=np.float32)
    use_qkb = bool(np.any(wq_b) or np.any(wk_b))
    use_vb = bool(np.any(wv_b))

    nc = _build(mask_mode, use_qkb, use_vb)

    xT = np.ascontiguousarray(x.reshape(T, DIM).T).astype(BF16)
    wqT = np.asarray(wq_w, dtype=np.float32).T.astype(BF16)  # [D, D]
    wkT = np.asarray(wk_w, dtype=np.float32).T.astype(BF16)
    wvT = np.asarray(wv_w, dtype=np.float32).T.astype(BF16)
    wo = np.asarray(wo_w, dtype=np.float32)

    in_maps = []
    for c in range(NCORES):
        sl = slice(HSL * c, HSL * (c + 1))
        im = {
            "xt": xT,
            "wqt": np.ascontiguousarray(wqT[:, sl]),
            "wkt": np.ascontiguousarray(wkT[:, sl]),
            "wvt": np.ascontiguousarray(wvT[:, sl]),
            "wot": np.ascontiguousarray(wo[:, sl].T).astype(BF16),
        }
        if use_qkb:
            im["qb"] = np.ascontiguousarray(wq_b[sl])
            im["kb"] = np.ascontiguousarray(wk_b[sl])
        if use_vb:
            im["vb"] = np.ascontiguousarray(wv_b[sl])
        if mask_mode == "general":
            im["maskt"] = maskT
        in_maps.append(im)

    res = run_bass_kernel_spmd(nc, in_maps, core_ids=list(range(NCORES)))
    LAST_RESULTS = res

    acc = np.zeros((DIM, T), dtype=np.float32)
    for r in res.results:
        acc += np.asarray(r["outT"], dtype=np.float32)
    out = acc.T + wo_b[None, :]
    return out.reshape(BSZ, SEQ, DIM).astype(np.float32)


# revision 27
# speedup vs baseline: 1.1712x; 1.0099x over previous
"""Trainium2 Bass kernel for nn_Attention_4930622456197.

Multi-head causal attention (B=2, S=2048, D=2048, 32 heads x head_dim 64)
with QKVO projections, tensor-parallel over heads across 8 NeuronCores
(4 heads per core).

Per-core plan (all matmul inputs bf16, f32 PSUM accumulation):
  Phase 1  QKV projections from host-pretransposed x^T [D, T]:
           Q^T, K^T in [128 (=2 heads x 64 dims), group, T] layout;
           V in natural [tok, head, 65] layout with a ones column
           appended (row 64 of V_aug.T) so the P@V matmul also produces
           the softmax denominators for free.
  Phase 2  Flash-style causal attention in score-transposed layout
           S^T[s, q] (scores never touch HBM).  exp on ScalarE with the
           1/sqrt(hd) scale folded in; no max-subtraction (scores are
           O(+-8) here, exp is safe in fp32->bf16).  The diagonal
           128x128 block of each strip is masked post-exp with a
           precomputed upper-triangular 0/1 tile.  O^T accumulates in
           PSUM over k-tiles; the 64 ones-columns of V_aug broadcast the
           softmax denominator to PSUM partitions 64..127, and 1/den is
           computed as exp(-ln(den)) on ScalarE.
  Phase 3  Row-parallel output projection producing a partial
           out^T [D, T]; host sums the 8 partials, adds wo_b.

  Emission interleaves the three phases: QKV chunks and output-projection
  tiles are "filler" thunks pumped between attention j-steps, keeping the
  PE array dense (HAM stays at 2.4 GHz) while ScalarE runs the softmax
  exps of the two in-flight head-pair streams.

The harness calls kernel(**inputs) with the full (unsharded) inputs and
expects the full [2, 2048, 2048] float32 output.
"""

import numpy as np
import ml_dtypes

BSZ, SEQ, DIM, NH = 2, 2048, 2048, 32
HD = DIM // NH            # 64
NCORES = 8
HPC = NH // NCORES        # 4 heads per core
HSL = HPC * HD            # 256 head-dims per core
T = BSZ * SEQ             # 4096 flattened tokens
SCALE = 1.0 / float(np.sqrt(HD))
BF16 = ml_dtypes.bfloat16

NKT = DIM // 128          # 16 contraction tiles over model dim
NCH = T // 512            # 8 token chunks of 512
NJ = SEQ // 128           # 16 k-tiles per sequence
NCK = SEQ // 512          # 4 q-chunks per sequence

# Output partial dtype: float32 is safest for the cross-core sum;
# bfloat16 halves the output DMA traffic.
OUT_BF16 = True

LAST_RESULTS = None       # BassKernelResults of the most recent run (for test.py)


# This walrus build caps EVERY instruction (HW-decoded and sequencer alike)
# at one sync-wait, so the legalizer splits excess waits regardless of opcode.
_SEQ_OPCODES = set()
_wc_counter = [0]


def _legalize_bir_waits(bir_bytes):
    """This container's walrus accepts only ONE sync-wait on HW-decoded
    instruction structs ("Too many sync wait commands" otherwise), but Tile
    freely emits 2-3 waits per instruction.  Split excess waits into
    standalone same-engine EventSemaphore instructions placed immediately
    before the instruction — the sequencer executes them in order, so the
    dependency semantics are identical."""
    import json as _json

    d = _json.loads(bir_bytes)
    n_split = 0
    for f in d.get("functions", []):
        for blk in f.get("blocks", []):
            out = []
            for ins in blk.get("instructions", []):
                si = ins.get("sync_info")
                waits = (si or {}).get("on_wait") or []
                if si is not None and len(waits) > 1 and \
                        ins.get("opcode") not in _SEQ_OPCODES:
                    for w in waits[:-1]:
                        _wc_counter[0] += 1
                        out.append({
                            "debug": ins.get("debug", 0),
                            "engine": ins["engine"],
                            "ins": [], "outs": [],
                            "name": f"I-wc{_wc_counter[0]}",
                            "opcode": "EventSemaphore",
                            "sync_info": {"on_wait": [w], "on_update": []},
                        })
                        n_split += 1
                    si["on_wait"] = waits[-1:]
                out.append(ins)
            blk["instructions"] = out
    if n_split:
        print(f"[kernel] wait-legalizer: split {n_split} excess waits")
    return _json.dumps(d).encode()


_hook_installed = [False]


def _install_compile_hook():
    """Route every BIR->NEFF compile in this process through the wait
    legalizer (both the direct bass_utils path and the bass2jax/axon path)."""
    if _hook_installed[0]:
        return
    import concourse.bass_utils as bu

    orig = bu.compile_bir_kernel

    def patched(bir_json, tmpdir, neff_name="file.neff"):
        return orig(_legalize_bir_waits(bir_json), tmpdir, neff_name=neff_name)

    bu.compile_bir_kernel = patched
    try:
        import concourse.bass2jax as b2j
        b2j.compile_bir_kernel = patched
    except Exception:
        pass
    _hook_installed[0] = True


def _build(mask_mode, use_qkb, use_vb, phases=(1, 2, 3)):
    """Builds the Bass program. mask_mode: 'causal' | 'none' | 'general'.
    phases: debug knob to emit only a subset of the pipeline."""
    import concourse.bass as bass
    import concourse.mybir as mybir
    import concourse.tile as tile
    from concourse.masks import make_upper_triangular

    dt = mybir.dt
    f32 = dt.float32
    bf16 = dt.bfloat16
    Exp = mybir.ActivationFunctionType.Exp
    Ln = mybir.ActivationFunctionType.Ln
    Identity = mybir.ActivationFunctionType.Identity
    out_dt = bf16 if OUT_BF16 else f32

    causal = mask_mode == "causal"

    nc = bass.Bass()
    xT_d = nc.dram_tensor("xt", [DIM, T], bf16, kind="ExternalInput")
    wqT_d = nc.dram_tensor("wqt", [DIM, HSL], bf16, kind="ExternalInput")
    wkT_d = nc.dram_tensor("wkt", [DIM, HSL], bf16, kind="ExternalInput")
    wvT_d = nc.dram_tensor("wvt", [DIM, HSL], bf16, kind="ExternalInput")
    woT_d = nc.dram_tensor("wot", [HSL, DIM], bf16, kind="ExternalInput")
    outT_d = nc.dram_tensor("outT", [DIM, T], out_dt, kind="ExternalOutput")
    qb_d = kb_d = vb_d = maskT_d = None
    if use_qkb:
        qb_d = nc.dram_tensor("qb", [HSL], f32, kind="ExternalInput")
        kb_d = nc.dram_tensor("kb", [HSL], f32, kind="ExternalInput")
    if use_vb:
        vb_d = nc.dram_tensor("vb", [HSL], f32, kind="ExternalInput")
    if mask_mode == "general":
        maskT_d = nc.dram_tensor("maskt", [SEQ, SEQ], f32, kind="ExternalInput")

    # 3-D views with 128-partition-major layout
    xT_ap = xT_d[:].rearrange("(kt p) t -> p kt t", p=128)
    wq_ap = wqT_d[:].rearrange("(kt p) m -> p kt m", p=128)
    wk_ap = wkT_d[:].rearrange("(kt p) m -> p kt m", p=128)
    wv_ap = wvT_d[:].rearrange("(kt p) m -> p kt m", p=128)
    wo_ap = woT_d[:].rearrange("(g p) n -> p g n", p=128)
    outT_ap = outT_d[:].rearrange("(ot p) t -> p ot t", p=128)

    with tile.TileContext(nc) as tc:
        with (
            tc.tile_pool(name="singles", bufs=1) as singles,
            tc.tile_pool(name="xload", bufs=3) as xload,
            tc.tile_pool(name="work", bufs=4) as work,
            tc.tile_pool(name="outp", bufs=4) as outp,
            tc.tile_pool(name="psum", bufs=2, space="PSUM") as psum,
            tc.tile_pool(name="otps", bufs=4, space="PSUM") as otps,
        ):
            # ---- resident tensors -------------------------------------
            wq_sb = singles.tile([128, NKT, HSL], bf16)
            wk_sb = singles.tile([128, NKT, HSL], bf16)
            wv_sb = singles.tile([128, NKT, HSL], bf16)
            wo_sb = singles.tile([128, 2, DIM], bf16)
            # wq is issued first so the very first Q-projection matmul can
            # start as early as possible; wk/wv/wo are issued from inside
            # qkv_thunks(0) right after the first x-chunk quarters.
            for q in range(4):
                ksl = slice(4 * q, 4 * q + 4)
                nc.sync.dma_start(out=wq_sb[:, ksl], in_=wq_ap[:, ksl])

            qt_sb = singles.tile([128, 2, T], bf16)
            kt_sb = singles.tile([128, 2, T], bf16)
            ctxT_sb = singles.tile([128, 2, T], bf16)
            # V with 64 ones-columns per head: the P@V matmul then writes the
            # softmax denominator to PSUM partitions 64..127 (a free
            # cross-partition broadcast).
            vaug_sb = singles.tile([128, T // 128, HPC, 2 * HD], bf16)
            nc.vector.memset(vaug_sb, 1.0)

            qb_sb = kb_sb = vb_bc = None
            if use_qkb:
                qb_sb = singles.tile([128, 2], f32)
                kb_sb = singles.tile([128, 2], f32)
                nc.sync.dma_start(out=qb_sb, in_=qb_d[:].rearrange("(g p) -> p g", p=128))
                nc.sync.dma_start(out=kb_sb, in_=kb_d[:].rearrange("(g p) -> p g", p=128))
            if use_vb:
                vb_bc = singles.tile([128, HSL], f32)
                nc.sync.dma_start(out=vb_bc, in_=vb_d[:].to_broadcast([128, HSL]))

            triu_sb = None
            if causal:
                triu_sb = singles.tile([128, 128], bf16)
                make_upper_triangular(nc, triu_sb, val=1.0, diag=True)

            # ---- emission units ---------------------------------------
            # QKV projections and the output projection are emitted as
            # "filler" thunks interleaved between attention j-steps, so PE
            # always has independent matmul work while ScalarE runs the
            # softmax exps of the in-flight attention streams.

            def qkv_thunks(ch):
                tsl = slice(ch * 512, (ch + 1) * 512)
                xt_box = []

                def load():
                    xt_ch = xload.tile([128, NKT, 512], bf16, tag="xt")
                    for q in range(4):
                        ksl = slice(4 * q, 4 * q + 4)
                        nc.sync.dma_start(out=xt_ch[:, ksl],
                                          in_=xT_ap[:, ksl, tsl])
                    xt_box.append(xt_ch)
                    if ch == 0:
                        for q in range(4):
                            ksl = slice(4 * q, 4 * q + 4)
                            nc.sync.dma_start(out=wk_sb[:, ksl],
                                              in_=wk_ap[:, ksl])
                            nc.sync.dma_start(out=wv_sb[:, ksl],
                                              in_=wv_ap[:, ksl])
                        nc.sync.dma_start(out=wo_sb, in_=wo_ap)
                yield load

                # both head-pair groups of one projection share a single
                # 2-bank PSUM tile (halves filler slot pressure in the st2
                # rotation) and evict with one instruction
                def qk_pair(w_sb, dst_sb, b_sb):
                    ps2 = psum.tile([128, 1024], f32, tag="st2", name="qk2")
                    for g in range(2):
                        for k in range(NKT):
                            nc.tensor.matmul(
                                ps2[:, g * 512:(g + 1) * 512],
                                lhsT=w_sb[:, k, g * 128:(g + 1) * 128],
                                rhs=xt_box[0][:, k, :],
                                start=(k == 0), stop=(k == NKT - 1))
                    if b_sb is not None:
                        for g in range(2):
                            nc.scalar.activation(
                                out=dst_sb[:, g, tsl],
                                in_=ps2[:, g * 512:(g + 1) * 512],
                                func=Identity, bias=b_sb[:, g:g + 1], scale=1.0)
                    else:
                        nc.vector.tensor_copy(
                            out=dst_sb[:, :, tsl],
                            in_=ps2.rearrange("p (g n) -> p g n", g=2))

                def v_pair(tp):
                    ps2 = psum.tile([128, 1024], f32, tag="st2", name="v2")
                    for i in range(2):
                        tt = 2 * tp + i
                        for k in range(NKT):
                            nc.tensor.matmul(
                                ps2[:, i * 512:i * 512 + HSL],
                                lhsT=xt_box[0][:, k, tt * 128:(tt + 1) * 128],
                                rhs=wv_sb[:, k, :],
                                start=(k == 0), stop=(k == NKT - 1))
                    tg0 = ch * 4 + 2 * tp
                    vdst = vaug_sb[:, tg0:tg0 + 2, :, 0:HD]
                    vsrc = ps2.rearrange("p (i n) -> p i n", i=2)[:, :, 0:HSL]
                    vsrc = vsrc.rearrange("p i (h m) -> p i h m", h=HPC)
                    if vb_bc is not None:
                        nc.vector.tensor_add(
                            out=vdst, in0=vsrc,
                            in1=vb_bc[:, None, :].to_broadcast(
                                [128, 2, HSL]).rearrange(
                                "p i (h m) -> p i h m", h=HPC))
                    else:
                        nc.vector.tensor_copy(out=vdst, in_=vsrc)

                import functools
                yield functools.partial(qk_pair, wq_sb, qt_sb, qb_sb)
                yield functools.partial(qk_pair, wk_sb, kt_sb, kb_sb)
                for tp in range(2):
                    yield functools.partial(v_pair, tp)

            def oproj_thunks(ch):
                import functools
                tsl = slice(ch * 512, (ch + 1) * 512)

                def o_pair(op):
                    ps2 = psum.tile([128, 1024], f32, tag="st2", name="o2")
                    for i in range(2):
                        o = 2 * op + i
                        for g2 in range(2):
                            nc.tensor.matmul(
                                ps2[:, i * 512:(i + 1) * 512],
                                lhsT=wo_sb[:, g2, o * 128:(o + 1) * 128],
                                rhs=ctxT_sb[:, g2, tsl],
                                start=(g2 == 0), stop=(g2 == 1))
                    osb = outp.tile([128, 2, 512], out_dt, tag="out_sb")
                    src2 = ps2.rearrange("p (i n) -> p i n", i=2)
                    if op % 2 == 0:
                        nc.vector.tensor_copy(out=osb, in_=src2)
                    else:
                        nc.scalar.copy(out=osb, in_=src2)
                    nc.sync.dma_start(
                        out=outT_ap[:, 2 * op:2 * op + 2, tsl], in_=osb)

                for op in range(DIM // 256):
                    yield functools.partial(o_pair, op)

            def pump(filler, n=1):
                for _ in range(n):
                    t = next(filler, None)
                    if t is None:
                        return False
                    t()
                return True

            def att_region(b, c, filler):
                """Attention for one (batch, q-chunk): head-pair streams g=0,1
                interleaved per j-step; O^T matmuls lag 2 steps; filler thunks
                are spread over the j-steps with a few reserved to bridge the
                region boundary while ScalarE drains the last exps."""
                thunks = list(filler)
                reserve = thunks[-9:]
                body = thunks[:-9]
                bi = [0]
                ots = {}
                for gg in range(2):
                    ots[gg, 0] = otps.tile([128, 512], f32, tag="ot", name="otA")
                    ots[gg, 1] = otps.tile([128, 512], f32, tag="ot", name="otB")
                jmax = 4 * c + 4 if causal else NJ
                pend = []

                def flush_ot(gg, j, qo, pt2):
                    for hh in range(2):
                        nc.tensor.matmul(
                            ots[gg, hh][:, qo:512],
                            lhsT=vaug_sb[:, b * NJ + j, 2 * gg + hh, :],
                            rhs=pt2[:, 512 * hh + qo:512 * hh + 512],
                            start=(j == 0), stop=(j == jmax - 1))

                for j in range(jmax):
                    qo = max(0, j * 128 - c * 512) if causal else 0
                    ssl = slice(b * SEQ + j * 128, b * SEQ + (j + 1) * 128)
                    qsl = slice(b * SEQ + c * 512 + qo, b * SEQ + (c + 1) * 512)
                    for gg in range(2):
                        st2 = psum.tile([128, 1024], f32, tag="st2", name="st2")
                        nc.tensor.matmul(
                            st2[:, qo:512], lhsT=kt_sb[0:64, gg, ssl],
                            rhs=qt_sb[0:64, gg, qsl],
                            start=True, stop=True, tile_position=(0, 0))
                        nc.tensor.matmul(
                            st2[:, 512 + qo:1024], lhsT=kt_sb[64:128, gg, ssl],
                            rhs=qt_sb[64:128, gg, qsl],
                            start=True, stop=True, tile_position=(64, 0))
                        if maskT_d is not None:
                            mt = work.tile([128, 512], f32, tag="mt")
                            nc.sync.dma_start(
                                out=mt,
                                in_=maskT_d[j * 128:(j + 1) * 128,
                                            c * 512:(c + 1) * 512])
                            for hh in range(2):
                                sl = slice(512 * hh, 512 * hh + 512)
                                nc.vector.tensor_add(
                                    out=st2[:, sl], in0=st2[:, sl], in1=mt)
                        pt2 = work.tile([128, 1024], bf16, tag="pt", bufs=8)
                        nc.scalar.activation(
                            out=pt2.rearrange("p (two n) -> p two n", two=2)[:, :, qo:512],
                            in_=st2.rearrange("p (two n) -> p two n", two=2)[:, :, qo:512],
                            func=Exp, scale=SCALE)
                        if causal and j * 128 >= c * 512:
                            dv = pt2.rearrange("p (two n) -> p two n", two=2)[:, :, qo:qo + 128]
                            nc.vector.tensor_mul(
                                out=dv, in0=dv,
                                in1=triu_sb[:, None, :].to_broadcast([128, 2, 128]))
                        pend.append((gg, j, qo, pt2))
                        while len(pend) > 6:
                            flush_ot(*pend.pop(0))
                    want = ((j + 1) * len(body) + jmax - 1) // jmax
                    while bi[0] < min(want, len(body)):
                        body[bi[0]]()
                        bi[0] += 1
                while pend:
                    flush_ot(*pend.pop(0))
                for t in reserve:
                    t()
                # chunk end: one f32 copy frees each accumulator slot; the
                # Ln/Exp reciprocal + multiply then run from SBUF overlapped
                # with the next region.
                for gg in range(2):
                    csl = slice(b * SEQ + c * 512, b * SEQ + (c + 1) * 512)
                    for hh in range(2):
                        ot = ots[gg, hh]
                        un = work.tile([128, 512], f32, tag="unctx")
                        nc.vector.tensor_copy(out=un, in_=ot)
                        rb = work.tile([64, 512], f32, tag="rb")
                        nc.scalar.activation(out=rb, in_=un[HD:2 * HD, :],
                                             func=Ln, scale=1.0)
                        nc.scalar.activation(out=rb, in_=rb,
                                             func=Exp, scale=-1.0)
                        nc.vector.tensor_mul(
                            out=ctxT_sb[hh * 64:(hh + 1) * 64, gg, csl],
                            in0=un[0:HD, :], in1=rb)

            # ---- schedule ---------------------------------------------
            from itertools import chain

            def drain(filler):
                while pump(filler):
                    pass

            if 1 in phases and 2 in phases and 3 in phases:
                drain(iter(qkv_thunks(0)))
                regions = [
                    (0, 0, qkv_thunks(1)),
                    (0, 1, qkv_thunks(2)),
                    (0, 2, qkv_thunks(3)),
                    (0, 3, chain(qkv_thunks(4), qkv_thunks(5))),
                    (1, 0, qkv_thunks(6)),
                    (1, 1, qkv_thunks(7)),
                    (1, 2, chain(oproj_thunks(0), oproj_thunks(1),
                                 oproj_thunks(5))),
                    (1, 3, chain(oproj_thunks(2), oproj_thunks(3),
                                 oproj_thunks(4), oproj_thunks(6))),
                ]
                for b, c, filler in regions:
                    att_region(b, c, filler)
                drain(iter(oproj_thunks(7)))
            else:
                # debug path: sequential phases
                if 1 in phases:
                    for ch in range(NCH):
                        drain(iter(qkv_thunks(ch)))
                if 2 in phases:
                    for b in range(BSZ):
                        for c in range(NCK):
                            att_region(b, c, ())
                if 3 in phases:
                    for ch in range(NCH):
                        drain(iter(oproj_thunks(ch)))

    return nc


def _classify_mask(mask):
    m = np.asarray(mask, dtype=np.float32).reshape(SEQ, SEQ)
    if not np.any(m):
        return "none", None
    lower_ok = not np.any(m[np.tril_indices(SEQ)])
    upper = m[np.triu_indices(SEQ, 1)]
    if lower_ok and np.all(np.isneginf(upper)):
        return "causal", None
    return "general", np.ascontiguousarray(m.T)


def kernel(x, start_pos, freqs_cis, mask, wq_w, wq_b, wk_w, wk_b,
           wv_w, wv_b, wo_w, wo_b):
    global LAST_RESULTS
    _install_compile_hook()
    from concourse.bass_utils import run_bass_kernel_spmd

    x = np.asarray(x, dtype=np.float32)
    mask_mode, maskT = _classify_mask(mask)
    wq_b = np.asarray(wq_b, dtype=np.float32)
    wk_b = np.asarray(wk_b, dtype=np.float32)
    wv_b = np.asarray(wv_b, dtype=np.float32)
    wo_b = np.asarray(wo_b, dtype=np.float32)
    use_qkb = bool(np.any(wq_b) or np.any(wk_b))
    use_vb = bool(np.any(wv_b))

    nc = _build(mask_mode, use_qkb, use_vb)

    xT = np.ascontiguousarray(x.reshape(T, DIM).T).astype(BF16)
    wqT = np.asarray(wq_w, dtype=np.float32).T.astype(BF16)  # [D, D]
    wkT = np.asarray(wk_w, dtype=np.float32).T.astype(BF16)
    wvT = np.asarray(wv_w, dtype=np.float32).T.astype(BF16)
    wo = np.asarray(wo_w, dtype=np.float32)

    in_maps = []
    for c in range(NCORES):
        sl = slice(HSL * c, HSL * (c + 1))
        im = {
            "xt": xT,
            "wqt": np.ascontiguousarray(wqT[:, sl]),
            "wkt": np.ascontiguousarray(wkT[:, sl]),
            "wvt": np.ascontiguousarray(wvT[:, sl]),
            "wot": np.ascontiguousarray(wo[:, sl].T).astype(BF16),
        }
        if use_qkb:
            im["qb"] = np.ascontiguousarray(wq_b[sl])
            im["kb"] = np.ascontiguousarray(wk_b[sl])
        if use_vb:
            im["vb"] = np.ascontiguousarray(wv_b[sl])
        if mask_mode == "general":
            im["maskt"] = maskT
        in_maps.append(im)

    res = run_bass_kernel_spmd(nc, in_maps, core_ids=list(range(NCORES)))
    LAST_RESULTS = res

    acc = np.zeros((DIM, T), dtype=np.float32)
    for r in res.results:
        acc += np.asarray(r["outT"], dtype=np.float32)
    out = acc.T + wo_b[None, :]
    return out.reshape(BSZ, SEQ, DIM).astype(np.float32)
